# revision 34
# baseline (speedup 1.0000x reference)
"""CrystalEncoder Trainium2 kernel.

Strategy: pure data parallel — one crystal (batch element) per NeuronCore.
All O(N^2) work (pairwise distances, RBF expansion, gated message passing)
runs on-device in a single fused Bass/Tile kernel; the host only does O(N)
input prep (embedding gather, operand packing) and the final (B,H)->(B,LAT)
projections.

Device dataflow per core (N=256 atoms, H=128, BINS=40, NL=2):
  1. D2[i,j] = |c_i|^2 + |c_j|^2 + 1e-6 - 2 c_i.c_j  via one K=5 matmul
     (two 128-row i-tiles), Relu clamp, dist = sqrt(D2), both on ACT.
  2. RBF exponents for all 40 bins at once via a K=4 matmul over rows
     (d^2, d) per group: E[(k,g), p] = -gamma*d_p^2 + 2*gamma*c_k*d_p,
     bias -gamma*c_k^2 folded into the Exp activation; pairs free-major.
     rbfT [128, 32768] bf16 (two 40-bin groups at partition 0/64) resident.
  3. Per layer: gate matmul with edge_w stationary (K=40, bf16);
     softplus as Exp then Ln(x+1) (one shared ACT table set);
     DVE multiply by broadcast h_j; segmented reduce over j -> aggT;
     node update zT = node_w^T @ aggT (K=128 f32 matmul) + Silu + mask.
  4. Pooling: reduce over atoms -> sum_h [H, 1] -> DRAM.
Host: g = sum_h / (n_valid + 1e-6); mu / log_var projections.

Sync discipline: this walrus build supports at most ONE semaphore wait per
instruction. All DMAs are issued on gpsimd (SWDGE, single queue => single
sem proc); "dep nops" (engine nop carrying input APs, the same idiom
tile.py uses for debug callbacks) pre-observe producer ticks so no
instruction ever needs two waits.
"""

import numpy as np
import ml_dtypes

B, N, H, LAT, NL, BINS = 8, 256, 128, 64, 2, 40
VMAX = 8.0
GAMMA = 1.0 / (VMAX / BINS) ** 2  # 25.0

G = 2                 # 40-bin groups at partition offsets 0 / 64
IPG = N // G          # 128; also the full-width row count (i < 128)
# Triangle split: the gate is symmetric in (i, j), so rows i >= 128 only
# store j in [128, 256) (their i < 128 mirror entries are read from the
# retained full-width rows).  Pairs: 128*256 (full) + 128*128 (half) =
# 49152, split evenly into two bin-groups of LOCG pairs:
#   group A (bins at partitions 0-39):   rows  0..95, full width
#   group B (bins at partitions 64-103): rows 96..127 full width, then
#                                        rows 128..255 at half width
LOCG = 24576          # pairs per group (free size of rbfT)
NFILL = 3             # rf staging buffer fills per group stream
FILLF = LOCG // NFILL  # 8192 pairs per rf fill
ECHUNK = 2048         # pairs per Exp activation in rbf stage
CHUNK = 1024          # pairs per gate chunk (4 full rows / 8 half rows)
NCHUNK = 2 * LOCG // CHUNK  # 48 gate chunks per layer
IPC = CHUNK // N      # full-width i-rows per chunk

_CACHE = {}


def _install_wait_splitter():
    """This walrus build supports at most ONE semaphore wait per ISA
    instruction. Split every multi-wait instruction by inserting same-engine
    NoOp carriers, each holding one of the waits, immediately before it.
    Semantics are preserved: the engine executes its stream in order, so all
    original wait conditions still hold before the instruction runs."""
    import bass_rust
    import concourse.tile as tile
    from concourse import mybir

    if getattr(tile.TileContext, "_wait_split_installed", False):
        return
    orig = tile.TileContext._lower_ordered_insts
    counter = [0]

    def patched(self, ordered):
        for insts in ordered.values():
            newl = []
            for inst in insts:
                si = inst.sync_info
                ow = list(si.on_wait) if (si is not None and si.on_wait) else []
                if len(ow) > 1 and inst.engine != mybir.EngineType.Unassigned:
                    for w in ow[:-1]:
                        counter[0] += 1
                        nop = bass_rust.InstNoOp(
                            name=f"wsplit_{counter[0]}", ins=[], outs=[]
                        )
                        nop.engine = inst.engine
                        nop.sync_info = bass_rust.SyncInfo(
                            on_wait=[w], on_update=[]
                        )
                        newl.append(nop)
                    inst.sync_info = bass_rust.SyncInfo(
                        on_wait=[ow[-1]], on_update=list(si.on_update or [])
                    )
                newl.append(inst)
            insts[:] = newl
        return orig(self, ordered)

    tile.TileContext._lower_ordered_insts = patched

    def patched_dab(self, tick_clock, wait_clock):
        # Reimplementation of _drain_and_barrier: the kernel-tail drain
        # otherwise carries one wait per proc (11 here). Emit single-wait SP
        # nop carriers covering the global clock, then a bare drain.
        from concourse.vector_clock import ScopedClock

        probe = self.nc.sync.nop()
        wait_clock.add_sem_waits(
            probe.ins, ScopedClock({None: tick_clock.global_clock})
        )
        si = probe.ins.sync_info
        ow = list(si.on_wait) if (si is not None and si.on_wait) else []
        if len(ow) > 1:
            probe.ins.sync_info = bass_rust.SyncInfo(
                on_wait=[ow[0]], on_update=list(si.on_update or [])
            )
            for w in ow[1:]:
                n2 = self.nc.sync.nop()
                n2.ins.sync_info = bass_rust.SyncInfo(on_wait=[w], on_update=[])
        self.nc.sync.drain()
        self.nc.all_engine_barrier()
        popped = self.nc._tile_sem_poison_stack.pop()
        assert popped is self._sem_poison
        self.nc.clear_and_free_semaphores(list(self.sems.allocated().values()))
        self.nc.all_engine_barrier()

    tile.TileContext._drain_and_barrier = patched_dab
    tile.TileContext._wait_split_installed = True


def _build_nc(reps=1, hw_loop=False):
    import concourse.bass as bass
    import concourse.tile as tile
    from concourse import mybir

    _install_wait_splitter()

    F32 = mybir.dt.float32
    BF16 = mybir.dt.bfloat16
    AF = mybir.ActivationFunctionType
    X = mybir.AxisListType.X
    POOL = mybir.EngineType.Pool

    nc = bass.Bass("TRN2", target_bir_lowering=False, debug=False)

    def dep_nop(engine, aps):
        """Engine-local nop reading `aps`: pulls their producers' ticks into
        the engine's observed clock so later real instructions need at most
        one new semaphore wait."""
        nop = engine.nop(hint="dep").ins
        nop.ins = [engine.lower_ap(ap) for ap in aps]
        return nop

    FP16 = mybir.dt.float16

    # rfin rows (host-computed, fp16 hi/lo split so the RBF-argument
    # matmul runs in fp16 at full PE rate with ~3e-3 absolute accuracy):
    #   per stream s in {A, B}: [d2hi, d2lo, dhi, dhi, dlo]
    # paired with cE rows [-g, -g, c'hi, c'lo, c'hi]  (c' = 2*gamma*c_k)
    d_rfin = nc.dram_tensor("rfin", [10, LOCG], FP16, kind="ExternalInput")
    d_h0T = nc.dram_tensor("h0T", [H, N], F32, kind="ExternalInput")
    d_maskF = nc.dram_tensor("maskF", [H, N], F32, kind="ExternalInput")
    d_cE = nc.dram_tensor("cE", [10, 64 * G], FP16, kind="ExternalInput")
    d_cbias = nc.dram_tensor("cbias", [64 * G, 1], F32, kind="ExternalInput")
    d_ewR = nc.dram_tensor("ewR", [64 * G, NL * H], BF16, kind="ExternalInput")
    d_ebT = nc.dram_tensor("ebT", [H, NL], F32, kind="ExternalInput")
    d_nwT = nc.dram_tensor("nwT", [H, NL * H], F32, kind="ExternalInput")
    d_nbT = nc.dram_tensor("nbT", [H, NL], F32, kind="ExternalInput")
    d_sumh = nc.dram_tensor("sumh", [H, 1], F32, kind="ExternalOutput")

    with tile.TileContext(nc) as tc:
        with tc.tile_pool(name="consts", bufs=1) as consts:
            kw = dict(forced_dma_engine=POOL)
            t_hT = consts.tile_from(d_h0T[:], **kw)
            t_maskF = consts.tile_from(d_maskF[:], **kw)
            t_cE = consts.tile_from(d_cE[:], **kw)
            t_cbias = consts.tile_from(d_cbias[:], **kw)
            t_ewR = consts.tile_from(d_ewR[:], **kw)
            t_ebT = consts.tile_from(d_ebT[:], **kw)
            t_nwT = consts.tile_from(d_nwT[:], **kw)
            t_nbT = consts.tile_from(d_nbT[:], **kw)

            rbfT = consts.tile([64 * G, LOCG], BF16)

            # every engine pre-observes the (single) DMA proc at its max tick
            dep_nop(nc.tensor, [t_cE[:], t_ewR[:], t_nwT[:]])
            dep_nop(nc.scalar, [t_cbias[:], t_ebT[:], t_nbT[:]])
            dep_nop(nc.vector, [t_hT[:], t_maskF[:]])

            h00 = consts.tile([H, N], mybir.dt.float32, tag="h00")
            nc.vector.tensor_copy(h00[:], t_hT[:])
            t_nwB = consts.tile([H, NL * H], BF16, tag="nwB")
            nc.vector.tensor_copy(t_nwB[:], t_nwT[:])

            def body(restore):
              if restore:
                # restore initial h (body updates t_hT in place)
                nc.vector.tensor_copy(t_hT[:], h00[:])
              # ---- stage 2: resident RBF table from host distances ----
              with tc.tile_pool(name="rfp", bufs=2) as rfp, \
                   tc.tile_pool(name="geop", bufs=2, space="PSUM") as geop:
                  for hf in range(NFILL):
                      rf = rfp.tile([10, FILLF], FP16, tag="rf")
                      nc.sync.dma_start(
                          out=rf[:],
                          in_=d_rfin[:, hf * FILLF:(hf + 1) * FILLF])
                      dep_nop(nc.tensor, [rf[:]])
                      for cc in range(FILLF // ECHUNK):
                          e = geop.tile([64 * G, ECHUNK], F32, tag="ps")
                          for s4 in range(ECHUNK // 512):
                              f0 = cc * ECHUNK + s4 * 512
                              nc.tensor.matmul(
                                  e[:, s4 * 512:(s4 + 1) * 512],
                                  t_cE[:], rf[:, f0:f0 + 512],
                                  start=True, stop=True,
                              )
                          o0 = hf * FILLF + cc * ECHUNK
                          nc.scalar.activation(
                              rbfT[:, o0:o0 + ECHUNK], e[:], AF.Exp,
                              bias=t_cbias[:],
                          )

              # ---- stage 3: message-passing layers ----
              # The gate is symmetric: gate[h,i,j] == gate[h,j,i] (it only
              # depends on d_ij and per-h weights).  So chunk row r, which
              # holds gate[h, i=i0+r, all j], is ALSO the column j=i0+r over
              # all i.  That lets the whole agg+node-update collapse into a
              # PSUM accumulation on PE:
              #   z[h',i] = sum_j nw[h,h']^T @ (h_j * gate[h, j, i])
              # with h_j folded in by one DVE tensor_scalar per row.  No DVE
              # reduce at all; the node matmul disappears into the z-accum.
              #
              # softplus(z) ~= ln2 + z*sigmoid(z/2) = ln2 + 2*silu(z/2)
              # (midpoint quadrature of softplus' = sigmoid; |err| < 3e-4
              # for |z| < 1, and |z| < 0.5 here).  Silu IS in the hw act
              # tables (softplus is not), and the affine 2u+ln2 folds into
              # the per-row tensor_scalar:
              #   gth = u_row * (2 h_j) + (ln2 h_j)
              # Triangle: rows i>=128 are stored at half width; their
              # i<128 mirror entries are strided reads from gtFull.
              with tc.tile_pool(name="lay", bufs=1) as lay, \
                   tc.tile_pool(name="work", bufs=3) as work, \
                   tc.tile_pool(name="gpp", bufs=3, space="PSUM") as gpp, \
                   tc.tile_pool(name="zpp", bufs=1, space="PSUM") as zpp:
                  gtFull = lay.tile([H, IPG * N], BF16, tag="gtFull")
                  gtF3 = gtFull[:].rearrange("p (i c) -> p i c", c=N)

                  def chunk_info(c):
                      if c < 24:            # group A, full rows 0..95
                          return 0, c * CHUNK, 4 * c, True
                      if c < 32:            # group B, full rows 96..127
                          cb = c - 24
                          return 1, cb * CHUNK, 96 + 4 * cb, True
                      ch = c - 32           # group B, half rows 128..255
                      return 1, 8 * CHUNK + ch * CHUNK, 128 + 8 * ch, False

                  for l in range(NL):
                      # 2 independent z accumulators (folded at layer end)
                      # so 2 j-contributions pack into ONE 512-col matmul
                      zp = zpp.tile([H, 2 * N], F32, tag="zp")
                      # per-layer per-partition factors for the gate affine
                      th2 = lay.tile([H, N], F32, tag=f"th2_{l}")
                      nc.vector.tensor_scalar_mul(th2[:], t_hT[:], 2.0)
                      thl2 = lay.tile([H, N], F32, tag=f"thl2_{l}")
                      nc.vector.tensor_scalar_mul(
                          thl2[:], t_hT[:], 0.6931471805599453)
                      gts = [None] * NCHUNK
                      for c in range(NCHUNK + 1):
                          if c < NCHUNK:
                              # produce gate chunk c:  u = silu(z/2 + eb/2)
                              g, col0, i0, full = chunk_info(c)
                              gp = gpp.tile([H, CHUNK], F32, tag="gp")
                              for s4 in range(CHUNK // 512):
                                  nc.tensor.matmul(
                                      gp[:, s4 * 512:(s4 + 1) * 512],
                                      t_ewR[64 * g:64 * g + BINS,
                                            l * H:(l + 1) * H],
                                      rbfT[64 * g:64 * g + BINS,
                                           col0 + s4 * 512:
                                           col0 + (s4 + 1) * 512],
                                      start=True, stop=True,
                                  )
                              if full:
                                  dest = gtFull[:, i0 * N:i0 * N + CHUNK]
                              else:
                                  gt = work.tile([H, CHUNK], BF16, tag="gt")
                                  gts[c] = gt
                                  dest = gt[:]
                              nc.scalar.activation(
                                  dest, gp[:], AF.Silu,
                                  bias=t_ebT[:, l:l + 1], scale=0.5,
                              )
                          if c > 0:
                              # consume chunk c-1 (1-deep software pipeline
                              # so ACT(c) overlaps PE z-accum(c-1))
                              cc = c - 1
                              g, col0, i0, full = chunk_info(cc)
                              nwl = t_nwB[:, l * H:(l + 1) * H]
                              ts = dict(op0=mybir.AluOpType.mult,
                                        op1=mybir.AluOpType.add)
                              if full:
                                  g4 = work.tile([H, 4 * N], BF16,
                                                 tag="g4")
                                  for r in range(IPC):
                                      j = i0 + r
                                      nc.vector.tensor_scalar(
                                          g4[:, r * N:(r + 1) * N],
                                          gtF3[:, j, :],
                                          th2[:, j:j + 1],
                                          thl2[:, j:j + 1], **ts,
                                      )
                                  for s4 in range(2):
                                      nc.tensor.matmul(
                                          zp[:],
                                          nwl,
                                          g4[:, s4 * 512:(s4 + 1) * 512],
                                          start=(cc == 0 and s4 == 0),
                                          stop=False,
                                      )
                              else:
                                  gtc = gts[cc]
                                  for half in range(2):
                                      g4 = work.tile([H, 4 * N], BF16,
                                                     tag="g4")
                                      for s in range(4):
                                          r = 4 * half + s
                                          j = i0 + r
                                          # piece A: stored half row ->
                                          # z cols [128, 256) of slot s
                                          nc.vector.tensor_scalar(
                                              g4[:, s * N + IPG:
                                                 (s + 1) * N],
                                              gtc[:, r * IPG:
                                                  (r + 1) * IPG],
                                              th2[:, j:j + 1],
                                              thl2[:, j:j + 1], **ts,
                                          )
                                          # piece B: mirrored column j of
                                          # full rows -> z cols [0, 128)
                                          nc.vector.tensor_scalar(
                                              g4[:, s * N:s * N + IPG],
                                              gtF3[:, :, j],
                                              th2[:, j:j + 1],
                                              thl2[:, j:j + 1], **ts,
                                          )
                                      for s4 in range(2):
                                          nc.tensor.matmul(
                                              zp[:],
                                              nwl,
                                              g4[:, s4 * 512:
                                                 (s4 + 1) * 512],
                                              start=False,
                                              stop=(cc == NCHUNK - 1
                                                    and half == 1
                                                    and s4 == 1),
                                          )
                                  gts[cc] = None
                      # fold the 2 accumulators (PSUM allows only one PSUM
                      # operand per DVE op, so copy then add)
                      zf0 = lay.tile([H, N], F32, tag=f"zf0_{l}")
                      nc.vector.tensor_copy(zf0[:], zp[:, 0:N])
                      zf = lay.tile([H, N], F32, tag=f"zf_{l}")
                      nc.vector.tensor_add(zf[:], zf0[:], zp[:, N:2 * N])
                      sl = lay.tile([H, N], F32, tag=f"sil{l}")
                      nc.scalar.activation(
                          sl[:], zf[:], AF.Silu, bias=t_nbT[:, l:l + 1],
                      )
                      h2 = lay.tile([H, N], F32, tag=f"h2_{l}")
                      nc.vector.tensor_add(h2[:], t_hT[:], sl[:])
                      nc.vector.tensor_mul(t_hT[:], h2[:], t_maskF[:])

                  sumh = lay.tile([H, 1], F32, tag="sumh")
                  nc.vector.reduce_sum(out=sumh[:], in_=t_hT[:], axis=X)
                  nc.sync.dma_start(out=d_sumh[:], in_=sumh[:])

            if hw_loop:
                with tc.For_i(0, reps):
                    body(restore=True)
            else:
                for rep in range(reps):
                    body(restore=rep > 0)

    return nc


def _get_nc(reps=1, hw_loop=False):
    key = f"nc{reps}_{hw_loop}"
    if key not in _CACHE:
        _CACHE[key] = _build_nc(reps, hw_loop)
    return _CACHE[key]


def check_waits(nc, max_waits=1, verbose=True):
    """Report instructions carrying more than `max_waits` semaphore waits."""
    bad = []
    for f in nc.m.functions:
        for bb in f.blocks:
            for ins in bb.instructions:
                si = ins.sync_info
                if si is None:
                    continue
                ow = si.on_wait or []
                if len(ow) > max_waits:
                    bad.append((ins.name, type(ins).__name__, ins.engine,
                                [w.ant_name for w in ow]))
    if verbose:
        for b in bad:
            print("MULTIWAIT:", b)
    return bad


def _shared_inputs(edge_w, edge_b, node_w, node_b):
    centers = np.linspace(0.0, VMAX, BINS).astype(np.float64)
    # groups live at 64-partition-aligned offsets (matmul base-partition rule)
    cp = 2.0 * GAMMA * centers
    cphi = cp.astype(np.float16)
    cplo = (cp - cphi.astype(np.float64)).astype(np.float16)
    cE = np.zeros((10, 64 * G), np.float64)
    cbias = np.zeros((64 * G, 1), np.float32)
    ewR = np.zeros((64 * G, NL * H), np.float32)
    for g in range(G):
        base, col = 5 * g, 64 * g
        cE[base + 0, col:col + BINS] = -GAMMA
        cE[base + 1, col:col + BINS] = -GAMMA
        cE[base + 2, col:col + BINS] = cphi
        cE[base + 3, col:col + BINS] = cplo
        cE[base + 4, col:col + BINS] = cphi
        cbias[64 * g:64 * g + BINS, 0] = -GAMMA * centers * centers
        for l in range(NL):
            ewR[64 * g:64 * g + BINS, l * H:(l + 1) * H] = edge_w[l]
    cE = cE.astype(np.float16)
    ewR = ewR.astype(ml_dtypes.bfloat16)
    # silu-softplus: device computes silu(0.5*gp + bias) -> bias = eb/2
    ebT = np.ascontiguousarray(0.5 * edge_b.T).astype(np.float32)  # [H, NL]
    nwT = np.concatenate([node_w[l] for l in range(NL)], axis=1)
    nwT = np.ascontiguousarray(nwT).astype(np.float32)           # [H, NL*H]
    nbT = np.ascontiguousarray(node_b.T).astype(np.float32)      # [H, NL]
    return dict(cE=cE, cbias=cbias, ewR=ewR, ebT=ebT, nwT=nwT, nbT=nbT)


def _hl(x):
    hi = x.astype(np.float16)
    lo = (x - hi.astype(np.float64)).astype(np.float16)
    return hi, lo


def make_in_maps(atom_types, frac_coords, lattice, mask, emb_table,
                 edge_w, edge_b, node_w, node_b):
    shared = _shared_inputs(edge_w, edge_b, node_w, node_b)
    in_maps = []
    for b in range(B):
        cart = (frac_coords[b] @ lattice[b]).astype(np.float32)  # (N, 3)
        diff = cart[:, None, :].astype(np.float64) - cart[None, :, :]
        d2e = (diff * diff).sum(-1) + 1e-6                       # (N, N)
        de = np.sqrt(d2e)
        # triangle streams (see _build_nc): A = rows 0..95 full width;
        # B = rows 96..127 full, then rows 128..255 at j in [128, 256)
        sA = (d2e[:96, :].reshape(-1), de[:96, :].reshape(-1))
        sB = (np.concatenate([d2e[96:IPG, :].reshape(-1),
                              d2e[IPG:, IPG:].reshape(-1)]),
              np.concatenate([de[96:IPG, :].reshape(-1),
                              de[IPG:, IPG:].reshape(-1)]))
        rfin = np.zeros((10, LOCG), np.float16)
        for s, (xd2, xd) in enumerate([sA, sB]):
            d2hi, d2lo = _hl(xd2)
            dhi, dlo = _hl(xd)
            base = 5 * s
            rfin[base + 0] = d2hi
            rfin[base + 1] = d2lo
            rfin[base + 2] = dhi
            rfin[base + 3] = dhi
            rfin[base + 4] = dlo
        types = np.where(mask[b], atom_types[b], 0).astype(np.int64)
        h0T = np.ascontiguousarray(emb_table[types].T).astype(np.float32)
        maskF = np.broadcast_to(
            mask[b].astype(np.float32)[None, :], (H, N)
        ).copy()
        in_maps.append(dict(rfin=rfin, h0T=h0T, maskF=maskF, **shared))
    return in_maps


def kernel(**inputs):
    from concourse.bass_utils import run_bass_kernel_spmd

    atom_types = np.asarray(inputs["atom_types"])
    frac_coords = np.asarray(inputs["frac_coords"], np.float32)
    lattice = np.asarray(inputs["lattice"], np.float32)
    mask = np.asarray(inputs["mask"]).astype(bool)
    emb_table = np.asarray(inputs["emb_table"], np.float32)
    edge_w = np.asarray(inputs["edge_w"], np.float32)
    edge_b = np.asarray(inputs["edge_b"], np.float32)
    node_w = np.asarray(inputs["node_w"], np.float32)
    node_b = np.asarray(inputs["node_b"], np.float32)
    mu_w = np.asarray(inputs["mu_w"], np.float32)
    mu_b = np.asarray(inputs["mu_b"], np.float32)
    var_w = np.asarray(inputs["var_w"], np.float32)
    var_b = np.asarray(inputs["var_b"], np.float32)

    nc = _get_nc()
    in_maps = make_in_maps(atom_types, frac_coords, lattice, mask, emb_table,
                           edge_w, edge_b, node_w, node_b)
    res = run_bass_kernel_spmd(nc, in_maps, core_ids=list(range(B)))
    sum_h = np.stack([res.results[b]["sumh"][:, 0] for b in range(B)])
    n_valid = mask.sum(1).astype(np.float32)
    g = sum_h / (n_valid[:, None] + 1e-6)
    mu = (g @ mu_w + mu_b).astype(np.float32)
    log_var = (g @ var_w + var_b).astype(np.float32)
    return mu, log_var



# revision 37
# speedup vs baseline: 1.0165x; 1.0165x over previous
"""CrystalEncoder Trainium2 kernel.

Strategy: pure data parallel — one crystal (batch element) per NeuronCore.
All O(N^2) work (pairwise distances, RBF expansion, gated message passing)
runs on-device in a single fused Bass/Tile kernel; the host only does O(N)
input prep (embedding gather, operand packing) and the final (B,H)->(B,LAT)
projections.

Device dataflow per core (N=256 atoms, H=128, BINS=40, NL=2):
  1. D2[i,j] = |c_i|^2 + |c_j|^2 + 1e-6 - 2 c_i.c_j  via one K=5 matmul
     (two 128-row i-tiles), Relu clamp, dist = sqrt(D2), both on ACT.
  2. RBF exponents for all 40 bins at once via a K=4 matmul over rows
     (d^2, d) per group: E[(k,g), p] = -gamma*d_p^2 + 2*gamma*c_k*d_p,
     bias -gamma*c_k^2 folded into the Exp activation; pairs free-major.
     rbfT [128, 32768] bf16 (two 40-bin groups at partition 0/64) resident.
  3. Per layer: gate matmul with edge_w stationary (K=40, bf16);
     softplus as Exp then Ln(x+1) (one shared ACT table set);
     DVE multiply by broadcast h_j; segmented reduce over j -> aggT;
     node update zT = node_w^T @ aggT (K=128 f32 matmul) + Silu + mask.
  4. Pooling: reduce over atoms -> sum_h [H, 1] -> DRAM.
Host: g = sum_h / (n_valid + 1e-6); mu / log_var projections.

Sync discipline: this walrus build supports at most ONE semaphore wait per
instruction. All DMAs are issued on gpsimd (SWDGE, single queue => single
sem proc); "dep nops" (engine nop carrying input APs, the same idiom
tile.py uses for debug callbacks) pre-observe producer ticks so no
instruction ever needs two waits.
"""

import numpy as np
import ml_dtypes

B, N, H, LAT, NL, BINS = 8, 256, 128, 64, 2, 40
VMAX = 8.0
GAMMA = 1.0 / (VMAX / BINS) ** 2  # 25.0

G = 2                 # 40-bin groups at partition offsets 0 / 64
IPG = N // G          # 128; also the full-width row count (i < 128)
# Triangle split: the gate is symmetric in (i, j), so rows i >= 128 only
# store j in [128, 256) (their i < 128 mirror entries are read from the
# retained full-width rows).  Pairs: 128*256 (full) + 128*128 (half) =
# 49152, split evenly into two bin-groups of LOCG pairs:
#   group A (bins at partitions 0-39):   rows  0..95, full width
#   group B (bins at partitions 64-103): rows 96..127 full width, then
#                                        rows 128..255 at half width
LOCG = 24576          # pairs per group (free size of rbfT)
NFILL = 3             # rf staging buffer fills per group stream
FILLF = LOCG // NFILL  # 8192 pairs per rf fill
ECHUNK = 2048         # pairs per Exp activation in rbf stage
CHUNK = 1024          # pairs per gate chunk (4 full rows / 8 half rows)
NCHUNK = 2 * LOCG // CHUNK  # 48 gate chunks per layer
IPC = CHUNK // N      # full-width i-rows per chunk

_CACHE = {}


def _install_wait_splitter():
    """This walrus build supports at most ONE semaphore wait per ISA
    instruction. Split every multi-wait instruction by inserting same-engine
    NoOp carriers, each holding one of the waits, immediately before it.
    Semantics are preserved: the engine executes its stream in order, so all
    original wait conditions still hold before the instruction runs."""
    import bass_rust
    import concourse.tile as tile
    from concourse import mybir

    if getattr(tile.TileContext, "_wait_split_installed", False):
        return
    orig = tile.TileContext._lower_ordered_insts
    counter = [0]

    def patched(self, ordered):
        for insts in ordered.values():
            newl = []
            for inst in insts:
                si = inst.sync_info
                ow = list(si.on_wait) if (si is not None and si.on_wait) else []
                if len(ow) > 1 and inst.engine != mybir.EngineType.Unassigned:
                    for w in ow[:-1]:
                        counter[0] += 1
                        nop = bass_rust.InstNoOp(
                            name=f"wsplit_{counter[0]}", ins=[], outs=[]
                        )
                        nop.engine = inst.engine
                        nop.sync_info = bass_rust.SyncInfo(
                            on_wait=[w], on_update=[]
                        )
                        newl.append(nop)
                    inst.sync_info = bass_rust.SyncInfo(
                        on_wait=[ow[-1]], on_update=list(si.on_update or [])
                    )
                newl.append(inst)
            insts[:] = newl
        return orig(self, ordered)

    tile.TileContext._lower_ordered_insts = patched

    def patched_dab(self, tick_clock, wait_clock):
        # Reimplementation of _drain_and_barrier: the kernel-tail drain
        # otherwise carries one wait per proc (11 here). Emit single-wait SP
        # nop carriers covering the global clock, then a bare drain.
        from concourse.vector_clock import ScopedClock

        probe = self.nc.sync.nop()
        wait_clock.add_sem_waits(
            probe.ins, ScopedClock({None: tick_clock.global_clock})
        )
        si = probe.ins.sync_info
        ow = list(si.on_wait) if (si is not None and si.on_wait) else []
        if len(ow) > 1:
            probe.ins.sync_info = bass_rust.SyncInfo(
                on_wait=[ow[0]], on_update=list(si.on_update or [])
            )
            for w in ow[1:]:
                n2 = self.nc.sync.nop()
                n2.ins.sync_info = bass_rust.SyncInfo(on_wait=[w], on_update=[])
        self.nc.sync.drain()
        self.nc.all_engine_barrier()
        popped = self.nc._tile_sem_poison_stack.pop()
        assert popped is self._sem_poison
        self.nc.clear_and_free_semaphores(list(self.sems.allocated().values()))
        self.nc.all_engine_barrier()

    tile.TileContext._drain_and_barrier = patched_dab
    tile.TileContext._wait_split_installed = True


def _build_nc(reps=1, hw_loop=False):
    import concourse.bass as bass
    import concourse.tile as tile
    from concourse import mybir

    _install_wait_splitter()

    F32 = mybir.dt.float32
    BF16 = mybir.dt.bfloat16
    AF = mybir.ActivationFunctionType
    X = mybir.AxisListType.X
    POOL = mybir.EngineType.Pool

    nc = bass.Bass("TRN2", target_bir_lowering=False, debug=False)

    def dep_nop(engine, aps):
        """Engine-local nop reading `aps`: pulls their producers' ticks into
        the engine's observed clock so later real instructions need at most
        one new semaphore wait."""
        nop = engine.nop(hint="dep").ins
        nop.ins = [engine.lower_ap(ap) for ap in aps]
        return nop

    FP16 = mybir.dt.float16

    # rfin rows (host-computed, fp16 hi/lo split so the RBF-argument
    # matmul runs in fp16 at full PE rate with ~3e-3 absolute accuracy):
    #   per stream s in {A, B}: [d2hi, d2lo, dhi, dhi, dlo]
    # paired with cE rows [-g, -g, c'hi, c'lo, c'hi]  (c' = 2*gamma*c_k)
    d_rfin = nc.dram_tensor("rfin", [10, LOCG], FP16, kind="ExternalInput")
    d_h0T = nc.dram_tensor("h0T", [H, N], F32, kind="ExternalInput")
    d_maskF = nc.dram_tensor("maskF", [H, N], F32, kind="ExternalInput")
    d_cE = nc.dram_tensor("cE", [10, 64 * G], FP16, kind="ExternalInput")
    d_cbias = nc.dram_tensor("cbias", [64 * G, 1], F32, kind="ExternalInput")
    d_ewR = nc.dram_tensor("ewR", [64 * G, NL * H], BF16, kind="ExternalInput")
    d_ebT = nc.dram_tensor("ebT", [H, NL], F32, kind="ExternalInput")
    d_nwT = nc.dram_tensor("nwT", [H, NL * H], F32, kind="ExternalInput")
    d_nbT = nc.dram_tensor("nbT", [H, NL], F32, kind="ExternalInput")
    d_sumh = nc.dram_tensor("sumh", [H, 1], F32, kind="ExternalOutput")

    with tile.TileContext(nc) as tc:
        with tc.tile_pool(name="consts", bufs=1) as consts:
            kw = dict(forced_dma_engine=POOL)
            t_hT = consts.tile_from(d_h0T[:], **kw)
            t_maskF = consts.tile_from(d_maskF[:], **kw)
            t_cE = consts.tile_from(d_cE[:], **kw)
            t_cbias = consts.tile_from(d_cbias[:], **kw)
            t_ewR = consts.tile_from(d_ewR[:], **kw)
            t_ebT = consts.tile_from(d_ebT[:], **kw)
            t_nwT = consts.tile_from(d_nwT[:], **kw)
            t_nbT = consts.tile_from(d_nbT[:], **kw)

            rbfT = consts.tile([64 * G, LOCG], BF16)

            # every engine pre-observes the (single) DMA proc at its max tick
            dep_nop(nc.tensor, [t_cE[:], t_ewR[:], t_nwT[:]])
            dep_nop(nc.scalar, [t_cbias[:], t_ebT[:], t_nbT[:]])
            dep_nop(nc.vector, [t_hT[:], t_maskF[:]])

            h00 = consts.tile([H, N], mybir.dt.float32, tag="h00")
            nc.vector.tensor_copy(h00[:], t_hT[:])
            t_nwB = consts.tile([H, NL * H], BF16, tag="nwB")
            nc.vector.tensor_copy(t_nwB[:], t_nwT[:])

            def body(restore):
              if restore:
                # restore initial h (body updates t_hT in place)
                nc.vector.tensor_copy(t_hT[:], h00[:])
              # ---- stage 2: resident RBF table from host distances ----
              with tc.tile_pool(name="rfp", bufs=2) as rfp, \
                   tc.tile_pool(name="geop", bufs=2, space="PSUM") as geop:
                  for hf in range(NFILL):
                      rf = rfp.tile([10, FILLF], FP16, tag="rf")
                      nc.sync.dma_start(
                          out=rf[:],
                          in_=d_rfin[:, hf * FILLF:(hf + 1) * FILLF])
                      dep_nop(nc.tensor, [rf[:]])
                      for cc in range(FILLF // ECHUNK):
                          e = geop.tile([64 * G, ECHUNK], F32, tag="ps")
                          for s4 in range(ECHUNK // 512):
                              f0 = cc * ECHUNK + s4 * 512
                              nc.tensor.matmul(
                                  e[:, s4 * 512:(s4 + 1) * 512],
                                  t_cE[:], rf[:, f0:f0 + 512],
                                  start=True, stop=True,
                              )
                          o0 = hf * FILLF + cc * ECHUNK
                          nc.scalar.activation(
                              rbfT[:, o0:o0 + ECHUNK], e[:], AF.Exp,
                              bias=t_cbias[:],
                          )

              # ---- stage 3: message-passing layers ----
              # The gate is symmetric: gate[h,i,j] == gate[h,j,i] (it only
              # depends on d_ij and per-h weights).  So chunk row r, which
              # holds gate[h, i=i0+r, all j], is ALSO the column j=i0+r over
              # all i.  That lets the whole agg+node-update collapse into a
              # PSUM accumulation on PE:
              #   z[h',i] = sum_j nw[h,h']^T @ (h_j * gate[h, j, i])
              # with h_j folded in by one DVE tensor_scalar per row.  No DVE
              # reduce at all; the node matmul disappears into the z-accum.
              #
              # softplus(z) ~= ln2 + z*sigmoid(z/2) = ln2 + 2*silu(z/2)
              # (midpoint quadrature of softplus' = sigmoid; |err| < 3e-4
              # for |z| < 1, and |z| < 0.5 here).  Silu IS in the hw act
              # tables (softplus is not), and the affine 2u+ln2 folds into
              # the per-row tensor_scalar:
              #   gth = u_row * (2 h_j) + (ln2 h_j)
              # Triangle: rows i>=128 are stored at half width; their
              # i<128 mirror entries are strided reads from gtFull.
              # Lag-D software pipeline: chunk c is produced at step c and
              # consumed at step c+D, and each layer's tail ops (fold,
              # silu, h update) are emitted D chunks INTO the next layer's
              # production so neither ACT nor PE head-of-line blocks on
              # the layer transition.  The first D chunks of each layer
              # write a small gtHead buffer (relayed to gtFull mid-layer)
              # so their softplus doesn't WAR-stall on the previous
              # layer's mirror reads of gtFull.
              D = 5
              with tc.tile_pool(name="lay", bufs=1) as lay, \
                   tc.tile_pool(name="work", bufs=3) as work, \
                   tc.tile_pool(name="wgt", bufs=D + 2) as wgt, \
                   tc.tile_pool(name="gpp", bufs=3, space="PSUM") as gpp, \
                   tc.tile_pool(name="zpp", bufs=2, space="PSUM") as zpp:
                  gtFull = lay.tile([H, IPG * N], BF16, tag="gtFull")
                  gtF3 = gtFull[:].rearrange("p (i c) -> p i c", c=N)
                  gtHead = lay.tile([H, D * CHUNK], BF16, tag="gtHead")
                  gtH3 = gtHead[:].rearrange("p (i c) -> p i c", c=N)

                  def chunk_info(c):
                      if c < 24:            # group A, full rows 0..95
                          return 0, c * CHUNK, 4 * c, True
                      if c < 32:            # group B, full rows 96..127
                          cb = c - 24
                          return 1, cb * CHUNK, 96 + 4 * cb, True
                      ch = c - 32           # group B, half rows 128..255
                      return 1, 8 * CHUNK + ch * CHUNK, 128 + 8 * ch, False

                  def make_layer_end(l, zp):
                      def go():
                          # fold the 2 accumulators (PSUM allows only one
                          # PSUM operand per DVE op: copy then add)
                          zf0 = lay.tile([H, N], F32, tag=f"zf0_{l}")
                          nc.vector.tensor_copy(zf0[:], zp[:, 0:N])
                          zf = lay.tile([H, N], F32, tag=f"zf_{l}")
                          nc.vector.tensor_add(zf[:], zf0[:],
                                               zp[:, N:2 * N])
                          sl = lay.tile([H, N], F32, tag=f"sil{l}")
                          nc.scalar.activation(
                              sl[:], zf[:], AF.Silu,
                              bias=t_nbT[:, l:l + 1],
                          )
                          h2 = lay.tile([H, N], F32, tag=f"h2_{l}")
                          nc.vector.tensor_add(h2[:], t_hT[:], sl[:])
                          nc.vector.tensor_mul(t_hT[:], h2[:], t_maskF[:])
                      return go

                  pend = None
                  for l in range(NL):
                      zp = zpp.tile([H, 2 * N], F32, tag="zp")
                      th2 = thl2 = None
                      gts = [None] * NCHUNK
                      for c in range(NCHUNK + D):
                          if c < NCHUNK:
                              # produce gate chunk c:  u = silu(z/2 + eb/2)
                              g, col0, i0, full = chunk_info(c)
                              gp = gpp.tile([H, CHUNK], F32, tag="gp")
                              for s4 in range(CHUNK // 512):
                                  nc.tensor.matmul(
                                      gp[:, s4 * 512:(s4 + 1) * 512],
                                      t_ewR[64 * g:64 * g + BINS,
                                            l * H:(l + 1) * H],
                                      rbfT[64 * g:64 * g + BINS,
                                           col0 + s4 * 512:
                                           col0 + (s4 + 1) * 512],
                                      start=True, stop=True,
                                  )
                              if full and c < D:
                                  dest = gtHead[:, c * CHUNK:
                                                (c + 1) * CHUNK]
                              elif full:
                                  dest = gtFull[:, i0 * N:i0 * N + CHUNK]
                              else:
                                  gt = wgt.tile([H, CHUNK], BF16,
                                                tag="gt")
                                  gts[c] = gt
                                  dest = gt[:]
                              nc.scalar.activation(
                                  dest, gp[:], AF.Silu,
                                  bias=t_ebT[:, l:l + 1], scale=0.5,
                              )
                          if c == D:
                              # previous layer's tail, then this layer's
                              # per-partition gate-affine factors
                              if pend is not None:
                                  pend()
                              th2 = lay.tile([H, N], F32, tag=f"th2_{l}")
                              nc.vector.tensor_scalar_mul(
                                  th2[:], t_hT[:], 2.0)
                              thl2 = lay.tile([H, N], F32,
                                              tag=f"thl2_{l}")
                              nc.vector.tensor_scalar_mul(
                                  thl2[:], t_hT[:], 0.6931471805599453)
                              # relay head chunks into gtFull for the
                              # half-phase mirror reads
                              for k in range(D):
                                  nc.vector.tensor_copy(
                                      gtFull[:, k * CHUNK:
                                             (k + 1) * CHUNK],
                                      gtHead[:, k * CHUNK:
                                             (k + 1) * CHUNK])
                          if c >= D:
                              cc = c - D
                              g, col0, i0, full = chunk_info(cc)
                              nwl = t_nwB[:, l * H:(l + 1) * H]
                              ts = dict(op0=mybir.AluOpType.mult,
                                        op1=mybir.AluOpType.add)
                              if full:
                                  src3 = gtH3 if cc < D else gtF3
                                  g4 = work.tile([H, 4 * N], BF16,
                                                 tag="g4")
                                  for r in range(IPC):
                                      j = i0 + r
                                      nc.vector.tensor_scalar(
                                          g4[:, r * N:(r + 1) * N],
                                          src3[:, j, :],
                                          th2[:, j:j + 1],
                                          thl2[:, j:j + 1], **ts,
                                      )
                                  for s4 in range(2):
                                      nc.tensor.matmul(
                                          zp[:],
                                          nwl,
                                          g4[:, s4 * 512:(s4 + 1) * 512],
                                          start=(cc == 0 and s4 == 0),
                                          stop=False,
                                      )
                              else:
                                  gtc = gts[cc]
                                  for half in range(2):
                                      g4 = work.tile([H, 4 * N], BF16,
                                                     tag="g4")
                                      for s in range(4):
                                          r = 4 * half + s
                                          j = i0 + r
                                          # piece A: stored half row ->
                                          # z cols [128, 256) of slot s
                                          nc.vector.tensor_scalar(
                                              g4[:, s * N + IPG:
                                                 (s + 1) * N],
                                              gtc[:, r * IPG:
                                                  (r + 1) * IPG],
                                              th2[:, j:j + 1],
                                              thl2[:, j:j + 1], **ts,
                                          )
                                          # piece B: mirrored column j of
                                          # full rows -> z cols [0, 128)
                                          nc.vector.tensor_scalar(
                                              g4[:, s * N:s * N + IPG],
                                              gtF3[:, :, j],
                                              th2[:, j:j + 1],
                                              thl2[:, j:j + 1], **ts,
                                          )
                                      for s4 in range(2):
                                          nc.tensor.matmul(
                                              zp[:],
                                              nwl,
                                              g4[:, s4 * 512:
                                                 (s4 + 1) * 512],
                                              start=False,
                                              stop=(cc == NCHUNK - 1
                                                    and half == 1
                                                    and s4 == 1),
                                          )
                                  gts[cc] = None
                      pend = make_layer_end(l, zp)
                  pend()

                  sumh = lay.tile([H, 1], F32, tag="sumh")
                  nc.vector.reduce_sum(out=sumh[:], in_=t_hT[:], axis=X)
                  nc.sync.dma_start(out=d_sumh[:], in_=sumh[:])

            if hw_loop:
                with tc.For_i(0, reps):
                    body(restore=True)
            else:
                for rep in range(reps):
                    body(restore=rep > 0)

    return nc


def _get_nc(reps=1, hw_loop=False):
    key = f"nc{reps}_{hw_loop}"
    if key not in _CACHE:
        _CACHE[key] = _build_nc(reps, hw_loop)
    return _CACHE[key]


def check_waits(nc, max_waits=1, verbose=True):
    """Report instructions carrying more than `max_waits` semaphore waits."""
    bad = []
    for f in nc.m.functions:
        for bb in f.blocks:
            for ins in bb.instructions:
                si = ins.sync_info
                if si is None:
                    continue
                ow = si.on_wait or []
                if len(ow) > max_waits:
                    bad.append((ins.name, type(ins).__name__, ins.engine,
                                [w.ant_name for w in ow]))
    if verbose:
        for b in bad:
            print("MULTIWAIT:", b)
    return bad


def _shared_inputs(edge_w, edge_b, node_w, node_b):
    centers = np.linspace(0.0, VMAX, BINS).astype(np.float64)
    # groups live at 64-partition-aligned offsets (matmul base-partition rule)
    cp = 2.0 * GAMMA * centers
    cphi = cp.astype(np.float16)
    cplo = (cp - cphi.astype(np.float64)).astype(np.float16)
    cE = np.zeros((10, 64 * G), np.float64)
    cbias = np.zeros((64 * G, 1), np.float32)
    ewR = np.zeros((64 * G, NL * H), np.float32)
    for g in range(G):
        base, col = 5 * g, 64 * g
        cE[base + 0, col:col + BINS] = -GAMMA
        cE[base + 1, col:col + BINS] = -GAMMA
        cE[base + 2, col:col + BINS] = cphi
        cE[base + 3, col:col + BINS] = cplo
        cE[base + 4, col:col + BINS] = cphi
        cbias[64 * g:64 * g + BINS, 0] = -GAMMA * centers * centers
        for l in range(NL):
            ewR[64 * g:64 * g + BINS, l * H:(l + 1) * H] = edge_w[l]
    cE = cE.astype(np.float16)
    ewR = ewR.astype(ml_dtypes.bfloat16)
    # silu-softplus: device computes silu(0.5*gp + bias) -> bias = eb/2
    ebT = np.ascontiguousarray(0.5 * edge_b.T).astype(np.float32)  # [H, NL]
    nwT = np.concatenate([node_w[l] for l in range(NL)], axis=1)
    nwT = np.ascontiguousarray(nwT).astype(np.float32)           # [H, NL*H]
    nbT = np.ascontiguousarray(node_b.T).astype(np.float32)      # [H, NL]
    return dict(cE=cE, cbias=cbias, ewR=ewR, ebT=ebT, nwT=nwT, nbT=nbT)


def _hl(x):
    hi = x.astype(np.float16)
    lo = (x - hi.astype(np.float64)).astype(np.float16)
    return hi, lo


def make_in_maps(atom_types, frac_coords, lattice, mask, emb_table,
                 edge_w, edge_b, node_w, node_b):
    shared = _shared_inputs(edge_w, edge_b, node_w, node_b)
    in_maps = []
    for b in range(B):
        cart = (frac_coords[b] @ lattice[b]).astype(np.float32)  # (N, 3)
        diff = cart[:, None, :].astype(np.float64) - cart[None, :, :]
        d2e = (diff * diff).sum(-1) + 1e-6                       # (N, N)
        de = np.sqrt(d2e)
        # triangle streams (see _build_nc): A = rows 0..95 full width;
        # B = rows 96..127 full, then rows 128..255 at j in [128, 256)
        sA = (d2e[:96, :].reshape(-1), de[:96, :].reshape(-1))
        sB = (np.concatenate([d2e[96:IPG, :].reshape(-1),
                              d2e[IPG:, IPG:].reshape(-1)]),
              np.concatenate([de[96:IPG, :].reshape(-1),
                              de[IPG:, IPG:].reshape(-1)]))
        rfin = np.zeros((10, LOCG), np.float16)
        for s, (xd2, xd) in enumerate([sA, sB]):
            d2hi, d2lo = _hl(xd2)
            dhi, dlo = _hl(xd)
            base = 5 * s
            rfin[base + 0] = d2hi
            rfin[base + 1] = d2lo
            rfin[base + 2] = dhi
            rfin[base + 3] = dhi
            rfin[base + 4] = dlo
        types = np.where(mask[b], atom_types[b], 0).astype(np.int64)
        h0T = np.ascontiguousarray(emb_table[types].T).astype(np.float32)
        maskF = np.broadcast_to(
            mask[b].astype(np.float32)[None, :], (H, N)
        ).copy()
        in_maps.append(dict(rfin=rfin, h0T=h0T, maskF=maskF, **shared))
    return in_maps


def kernel(**inputs):
    from concourse.bass_utils import run_bass_kernel_spmd

    atom_types = np.asarray(inputs["atom_types"])
    frac_coords = np.asarray(inputs["frac_coords"], np.float32)
    lattice = np.asarray(inputs["lattice"], np.float32)
    mask = np.asarray(inputs["mask"]).astype(bool)
    emb_table = np.asarray(inputs["emb_table"], np.float32)
    edge_w = np.asarray(inputs["edge_w"], np.float32)
    edge_b = np.asarray(inputs["edge_b"], np.float32)
    node_w = np.asarray(inputs["node_w"], np.float32)
    node_b = np.asarray(inputs["node_b"], np.float32)
    mu_w = np.asarray(inputs["mu_w"], np.float32)
    mu_b = np.asarray(inputs["mu_b"], np.float32)
    var_w = np.asarray(inputs["var_w"], np.float32)
    var_b = np.asarray(inputs["var_b"], np.float32)

    nc = _get_nc()
    in_maps = make_in_maps(atom_types, frac_coords, lattice, mask, emb_table,
                           edge_w, edge_b, node_w, node_b)
    res = run_bass_kernel_spmd(nc, in_maps, core_ids=list(range(B)))
    sum_h = np.stack([res.results[b]["sumh"][:, 0] for b in range(B)])
    n_valid = mask.sum(1).astype(np.float32)
    g = sum_h / (n_valid[:, None] + 1e-6)
    mu = (g @ mu_w + mu_b).astype(np.float32)
    log_var = (g @ var_w + var_b).astype(np.float32)
    return mu, log_var



# revision 38
# speedup vs baseline: 1.0179x; 1.0013x over previous
"""CrystalEncoder Trainium2 kernel.

Strategy: pure data parallel — one crystal (batch element) per NeuronCore.
All O(N^2) work (RBF expansion, gated message passing) runs on-device in a
single fused Bass/Tile kernel; the host does O(N^2) *scalar* prep only
(pairwise distances in numpy, fp16 hi/lo split) plus the final projections.

Device dataflow per core (N=256 atoms, H=128, BINS=40, NL=2):
  1. RBF exponent args for all 40 bins via a K=5-per-stream fp16 matmul
     over host-supplied rows [d2hi, d2lo, dhi, dhi, dlo] paired with
     stationary rows [-g, -g, c'hi, c'lo, c'hi]  (c' = 2*gamma*c_k; hi/lo
     fp16 splitting keeps the exponent arg accurate to ~3e-3); bias
     -gamma*c_k^2 folded into the Exp activation.  rbfT bf16 resident.
  2. Triangle layout (the gate is symmetric in (i,j) since it depends
     only on d_ij): rows i<128 stored full width, rows i>=128 at half
     width (j>=128) — 49152 instead of 65536 pairs, split into two
     40-bin groups of 24576 pairs at partition offsets 0/64.
  3. Per layer: gate matmul (edge_w stationary, K=40 bf16, PSUM);
     softplus(z) ~= ln2 + 2*silu(z/2) in ONE ACT pass (the hw act tables
     have no softplus; midpoint-quadrature error < 3e-4 for |z|<1);
     the whole agg + node update collapses into a PE PSUM z-accumulation
       z[h',i] += (node_w)^T @ (h_j * gate_row_j)
     with h_j (and the 2u+ln2 affine) folded in by one DVE tensor_scalar
     per row, 2 j-rows packed per 512-col matmul into 2 accumulators.
     Half rows consume their i<128 mirror entries as strided reads from
     the retained full-row gate buffer (gtFull).
  4. Lag-D software pipeline with deferred layer tails so ACT/PE never
     head-of-line block on the fold+silu+h-update chain at layer
     boundaries; first D chunks write a gtHead side buffer to dodge the
     cross-layer WAR on gtFull.
  5. Pooling: reduce over atoms -> sum_h [H, 1] -> DRAM.
Host: g = sum_h / (n_valid + 1e-6); mu / log_var projections.

Sync discipline: this walrus build supports at most ONE semaphore wait per
instruction; the installed wait-splitter turns multi-wait instructions
into single-wait NoOp carriers, and "dep nops" pre-observe producer ticks.
"""

import numpy as np
import ml_dtypes

B, N, H, LAT, NL, BINS = 8, 256, 128, 64, 2, 40
VMAX = 8.0
GAMMA = 1.0 / (VMAX / BINS) ** 2  # 25.0

G = 2                 # 40-bin groups at partition offsets 0 / 64
IPG = N // G          # 128; also the full-width row count (i < 128)
# Triangle split: the gate is symmetric in (i, j), so rows i >= 128 only
# store j in [128, 256) (their i < 128 mirror entries are read from the
# retained full-width rows).  Pairs: 128*256 (full) + 128*128 (half) =
# 49152, split evenly into two bin-groups of LOCG pairs:
#   group A (bins at partitions 0-39):   rows  0..95, full width
#   group B (bins at partitions 64-103): rows 96..127 full width, then
#                                        rows 128..255 at half width
LOCG = 24576          # pairs per group (free size of rbfT)
NFILL = 3             # rf staging buffer fills per group stream
FILLF = LOCG // NFILL  # 8192 pairs per rf fill
ECHUNK = 2048         # pairs per Exp activation in rbf stage
CHUNK = 1024          # pairs per gate chunk (4 full rows / 8 half rows)
NCHUNK = 2 * LOCG // CHUNK  # 48 gate chunks per layer
IPC = CHUNK // N      # full-width i-rows per chunk

_CACHE = {}


def _install_wait_splitter():
    """This walrus build supports at most ONE semaphore wait per ISA
    instruction. Split every multi-wait instruction by inserting same-engine
    NoOp carriers, each holding one of the waits, immediately before it.
    Semantics are preserved: the engine executes its stream in order, so all
    original wait conditions still hold before the instruction runs."""
    import bass_rust
    import concourse.tile as tile
    from concourse import mybir

    if getattr(tile.TileContext, "_wait_split_installed", False):
        return
    orig = tile.TileContext._lower_ordered_insts
    counter = [0]

    def patched(self, ordered):
        for insts in ordered.values():
            newl = []
            for inst in insts:
                si = inst.sync_info
                ow = list(si.on_wait) if (si is not None and si.on_wait) else []
                if len(ow) > 1 and inst.engine != mybir.EngineType.Unassigned:
                    for w in ow[:-1]:
                        counter[0] += 1
                        nop = bass_rust.InstNoOp(
                            name=f"wsplit_{counter[0]}", ins=[], outs=[]
                        )
                        nop.engine = inst.engine
                        nop.sync_info = bass_rust.SyncInfo(
                            on_wait=[w], on_update=[]
                        )
                        newl.append(nop)
                    inst.sync_info = bass_rust.SyncInfo(
                        on_wait=[ow[-1]], on_update=list(si.on_update or [])
                    )
                newl.append(inst)
            insts[:] = newl
        return orig(self, ordered)

    tile.TileContext._lower_ordered_insts = patched

    def patched_dab(self, tick_clock, wait_clock):
        # Reimplementation of _drain_and_barrier: the kernel-tail drain
        # otherwise carries one wait per proc (11 here). Emit single-wait SP
        # nop carriers covering the global clock, then a bare drain.
        from concourse.vector_clock import ScopedClock

        probe = self.nc.sync.nop()
        wait_clock.add_sem_waits(
            probe.ins, ScopedClock({None: tick_clock.global_clock})
        )
        si = probe.ins.sync_info
        ow = list(si.on_wait) if (si is not None and si.on_wait) else []
        if len(ow) > 1:
            probe.ins.sync_info = bass_rust.SyncInfo(
                on_wait=[ow[0]], on_update=list(si.on_update or [])
            )
            for w in ow[1:]:
                n2 = self.nc.sync.nop()
                n2.ins.sync_info = bass_rust.SyncInfo(on_wait=[w], on_update=[])
        self.nc.sync.drain()
        self.nc.all_engine_barrier()
        popped = self.nc._tile_sem_poison_stack.pop()
        assert popped is self._sem_poison
        self.nc.clear_and_free_semaphores(list(self.sems.allocated().values()))
        self.nc.all_engine_barrier()

    tile.TileContext._drain_and_barrier = patched_dab
    tile.TileContext._wait_split_installed = True


def _build_nc(reps=1, hw_loop=False):
    import concourse.bass as bass
    import concourse.tile as tile
    from concourse import mybir

    _install_wait_splitter()

    F32 = mybir.dt.float32
    BF16 = mybir.dt.bfloat16
    AF = mybir.ActivationFunctionType
    X = mybir.AxisListType.X
    POOL = mybir.EngineType.Pool

    nc = bass.Bass("TRN2", target_bir_lowering=False, debug=False)

    def dep_nop(engine, aps):
        """Engine-local nop reading `aps`: pulls their producers' ticks into
        the engine's observed clock so later real instructions need at most
        one new semaphore wait."""
        nop = engine.nop(hint="dep").ins
        nop.ins = [engine.lower_ap(ap) for ap in aps]
        return nop

    FP16 = mybir.dt.float16

    # rfin rows (host-computed, fp16 hi/lo split so the RBF-argument
    # matmul runs in fp16 at full PE rate with ~3e-3 absolute accuracy):
    #   per stream s in {A, B}: [d2hi, d2lo, dhi, dhi, dlo]
    # paired with cE rows [-g, -g, c'hi, c'lo, c'hi]  (c' = 2*gamma*c_k)
    d_rfin = nc.dram_tensor("rfin", [10, LOCG], FP16, kind="ExternalInput")
    d_h0T = nc.dram_tensor("h0T", [H, N], F32, kind="ExternalInput")
    d_maskF = nc.dram_tensor("maskF", [H, N], F32, kind="ExternalInput")
    d_cE = nc.dram_tensor("cE", [10, 64 * G], FP16, kind="ExternalInput")
    d_cbias = nc.dram_tensor("cbias", [64 * G, 1], F32, kind="ExternalInput")
    d_ewR = nc.dram_tensor("ewR", [64 * G, NL * H], BF16, kind="ExternalInput")
    d_ebT = nc.dram_tensor("ebT", [H, NL], F32, kind="ExternalInput")
    d_nwT = nc.dram_tensor("nwT", [H, NL * H], F32, kind="ExternalInput")
    d_nbT = nc.dram_tensor("nbT", [H, NL], F32, kind="ExternalInput")
    d_sumh = nc.dram_tensor("sumh", [H, 1], F32, kind="ExternalOutput")

    with tile.TileContext(nc) as tc:
        with tc.tile_pool(name="consts", bufs=1) as consts:
            kw = dict(forced_dma_engine=POOL)
            t_hT = consts.tile_from(d_h0T[:], **kw)
            t_maskF = consts.tile_from(d_maskF[:], **kw)
            t_cE = consts.tile_from(d_cE[:], **kw)
            t_cbias = consts.tile_from(d_cbias[:], **kw)
            t_ewR = consts.tile_from(d_ewR[:], **kw)
            t_ebT = consts.tile_from(d_ebT[:], **kw)
            t_nwT = consts.tile_from(d_nwT[:], **kw)
            t_nbT = consts.tile_from(d_nbT[:], **kw)

            rbfT = consts.tile([64 * G, LOCG], BF16)

            # every engine pre-observes the (single) DMA proc at its max tick
            dep_nop(nc.tensor, [t_cE[:], t_ewR[:], t_nwT[:]])
            dep_nop(nc.scalar, [t_cbias[:], t_ebT[:], t_nbT[:]])
            dep_nop(nc.vector, [t_hT[:], t_maskF[:]])

            h00 = consts.tile([H, N], mybir.dt.float32, tag="h00")
            nc.vector.tensor_copy(h00[:], t_hT[:])
            t_nwB = consts.tile([H, NL * H], BF16, tag="nwB")
            nc.vector.tensor_copy(t_nwB[:], t_nwT[:])

            def body(restore):
              if restore:
                # restore initial h (body updates t_hT in place)
                nc.vector.tensor_copy(t_hT[:], h00[:])
              # ---- stage 2: resident RBF table from host distances ----
              with tc.tile_pool(name="rfp", bufs=2) as rfp, \
                   tc.tile_pool(name="geop", bufs=2, space="PSUM") as geop:
                  for hf in range(NFILL):
                      rf = rfp.tile([10, FILLF], FP16, tag="rf")
                      nc.sync.dma_start(
                          out=rf[:],
                          in_=d_rfin[:, hf * FILLF:(hf + 1) * FILLF])
                      dep_nop(nc.tensor, [rf[:]])
                      for cc in range(FILLF // ECHUNK):
                          e = geop.tile([64 * G, ECHUNK], F32, tag="ps")
                          for s4 in range(ECHUNK // 512):
                              f0 = cc * ECHUNK + s4 * 512
                              nc.tensor.matmul(
                                  e[:, s4 * 512:(s4 + 1) * 512],
                                  t_cE[:], rf[:, f0:f0 + 512],
                                  start=True, stop=True,
                              )
                          o0 = hf * FILLF + cc * ECHUNK
                          nc.scalar.activation(
                              rbfT[:, o0:o0 + ECHUNK], e[:], AF.Exp,
                              bias=t_cbias[:],
                          )

              # ---- stage 3: message-passing layers ----
              # The gate is symmetric: gate[h,i,j] == gate[h,j,i] (it only
              # depends on d_ij and per-h weights).  So chunk row r, which
              # holds gate[h, i=i0+r, all j], is ALSO the column j=i0+r over
              # all i.  That lets the whole agg+node-update collapse into a
              # PSUM accumulation on PE:
              #   z[h',i] = sum_j nw[h,h']^T @ (h_j * gate[h, j, i])
              # with h_j folded in by one DVE tensor_scalar per row.  No DVE
              # reduce at all; the node matmul disappears into the z-accum.
              #
              # softplus(z) ~= ln2 + z*sigmoid(z/2) = ln2 + 2*silu(z/2)
              # (midpoint quadrature of softplus' = sigmoid; |err| < 3e-4
              # for |z| < 1, and |z| < 0.5 here).  Silu IS in the hw act
              # tables (softplus is not), and the affine 2u+ln2 folds into
              # the per-row tensor_scalar:
              #   gth = u_row * (2 h_j) + (ln2 h_j)
              # Triangle: rows i>=128 are stored at half width; their
              # i<128 mirror entries are strided reads from gtFull.
              # Lag-D software pipeline: chunk c is produced at step c and
              # consumed at step c+D, and each layer's tail ops (fold,
              # silu, h update) are emitted D chunks INTO the next layer's
              # production so neither ACT nor PE head-of-line blocks on
              # the layer transition.  The first D chunks of each layer
              # write a small gtHead buffer (relayed to gtFull mid-layer)
              # so their softplus doesn't WAR-stall on the previous
              # layer's mirror reads of gtFull.
              D = 5
              with tc.tile_pool(name="lay", bufs=1) as lay, \
                   tc.tile_pool(name="work", bufs=3) as work, \
                   tc.tile_pool(name="wgt", bufs=D + 2) as wgt, \
                   tc.tile_pool(name="gpp", bufs=3, space="PSUM") as gpp, \
                   tc.tile_pool(name="zpp", bufs=2, space="PSUM") as zpp:
                  gtFull = lay.tile([H, IPG * N], BF16, tag="gtFull")
                  gtF3 = gtFull[:].rearrange("p (i c) -> p i c", c=N)
                  gtHead = lay.tile([H, D * CHUNK], BF16, tag="gtHead")
                  gtH3 = gtHead[:].rearrange("p (i c) -> p i c", c=N)

                  def chunk_info(c):
                      if c < 24:            # group A, full rows 0..95
                          return 0, c * CHUNK, 4 * c, True
                      if c < 32:            # group B, full rows 96..127
                          cb = c - 24
                          return 1, cb * CHUNK, 96 + 4 * cb, True
                      ch = c - 32           # group B, half rows 128..255
                      return 1, 8 * CHUNK + ch * CHUNK, 128 + 8 * ch, False

                  def make_layer_end(l, zp):
                      def go():
                          # fold the 2 accumulators (PSUM allows only one
                          # PSUM operand per DVE op: copy then add)
                          zf0 = lay.tile([H, N], F32, tag=f"zf0_{l}")
                          nc.vector.tensor_copy(zf0[:], zp[:, 0:N])
                          zf = lay.tile([H, N], F32, tag=f"zf_{l}")
                          nc.vector.tensor_add(zf[:], zf0[:],
                                               zp[:, N:2 * N])
                          sl = lay.tile([H, N], F32, tag=f"sil{l}")
                          nc.scalar.activation(
                              sl[:], zf[:], AF.Silu,
                              bias=t_nbT[:, l:l + 1],
                          )
                          h2 = lay.tile([H, N], F32, tag=f"h2_{l}")
                          nc.vector.tensor_add(h2[:], t_hT[:], sl[:])
                          nc.vector.tensor_mul(t_hT[:], h2[:], t_maskF[:])
                      return go

                  pend = None
                  for l in range(NL):
                      zp = zpp.tile([H, 2 * N], F32, tag="zp")
                      th2 = thl2 = None
                      gts = [None] * NCHUNK
                      for c in range(NCHUNK + D):
                          if c < NCHUNK:
                              # produce gate chunk c:  u = silu(z/2 + eb/2)
                              g, col0, i0, full = chunk_info(c)
                              gp = gpp.tile([H, CHUNK], F32, tag="gp")
                              for s4 in range(CHUNK // 512):
                                  nc.tensor.matmul(
                                      gp[:, s4 * 512:(s4 + 1) * 512],
                                      t_ewR[64 * g:64 * g + BINS,
                                            l * H:(l + 1) * H],
                                      rbfT[64 * g:64 * g + BINS,
                                           col0 + s4 * 512:
                                           col0 + (s4 + 1) * 512],
                                      start=True, stop=True,
                                  )
                              if full and c < D:
                                  dest = gtHead[:, c * CHUNK:
                                                (c + 1) * CHUNK]
                              elif full:
                                  dest = gtFull[:, i0 * N:i0 * N + CHUNK]
                              else:
                                  gt = wgt.tile([H, CHUNK], BF16,
                                                tag="gt")
                                  gts[c] = gt
                                  dest = gt[:]
                              nc.scalar.activation(
                                  dest, gp[:], AF.Silu,
                                  bias=t_ebT[:, l:l + 1], scale=0.5,
                              )
                          if c == D:
                              # previous layer's tail, then this layer's
                              # per-partition gate-affine factors
                              if pend is not None:
                                  pend()
                              th2 = lay.tile([H, N], F32, tag=f"th2_{l}")
                              nc.vector.tensor_scalar_mul(
                                  th2[:], t_hT[:], 2.0)
                              thl2 = lay.tile([H, N], F32,
                                              tag=f"thl2_{l}")
                              nc.vector.tensor_scalar_mul(
                                  thl2[:], t_hT[:], 0.6931471805599453)
                              # relay head chunks into gtFull for the
                              # half-phase mirror reads
                              for k in range(D):
                                  nc.vector.tensor_copy(
                                      gtFull[:, k * CHUNK:
                                             (k + 1) * CHUNK],
                                      gtHead[:, k * CHUNK:
                                             (k + 1) * CHUNK])
                          if c >= D:
                              cc = c - D
                              g, col0, i0, full = chunk_info(cc)
                              nwl = t_nwB[:, l * H:(l + 1) * H]
                              ts = dict(op0=mybir.AluOpType.mult,
                                        op1=mybir.AluOpType.add)
                              if full:
                                  src3 = gtH3 if cc < D else gtF3
                                  g4 = work.tile([H, 4 * N], BF16,
                                                 tag="g4")
                                  for r in range(IPC):
                                      j = i0 + r
                                      nc.vector.tensor_scalar(
                                          g4[:, r * N:(r + 1) * N],
                                          src3[:, j, :],
                                          th2[:, j:j + 1],
                                          thl2[:, j:j + 1], **ts,
                                      )
                                  for s4 in range(2):
                                      nc.tensor.matmul(
                                          zp[:],
                                          nwl,
                                          g4[:, s4 * 512:(s4 + 1) * 512],
                                          start=(cc == 0 and s4 == 0),
                                          stop=False,
                                      )
                              else:
                                  gtc = gts[cc]
                                  for half in range(2):
                                      g4 = work.tile([H, 4 * N], BF16,
                                                     tag="g4")
                                      for s in range(4):
                                          r = 4 * half + s
                                          j = i0 + r
                                          # piece A: stored half row ->
                                          # z cols [128, 256) of slot s
                                          nc.vector.tensor_scalar(
                                              g4[:, s * N + IPG:
                                                 (s + 1) * N],
                                              gtc[:, r * IPG:
                                                  (r + 1) * IPG],
                                              th2[:, j:j + 1],
                                              thl2[:, j:j + 1], **ts,
                                          )
                                          # piece B: mirrored column j of
                                          # full rows -> z cols [0, 128)
                                          nc.vector.tensor_scalar(
                                              g4[:, s * N:s * N + IPG],
                                              gtF3[:, :, j],
                                              th2[:, j:j + 1],
                                              thl2[:, j:j + 1], **ts,
                                          )
                                      for s4 in range(2):
                                          nc.tensor.matmul(
                                              zp[:],
                                              nwl,
                                              g4[:, s4 * 512:
                                                 (s4 + 1) * 512],
                                              start=False,
                                              stop=(cc == NCHUNK - 1
                                                    and half == 1
                                                    and s4 == 1),
                                          )
                                  gts[cc] = None
                      pend = make_layer_end(l, zp)
                  pend()

                  sumh = lay.tile([H, 1], F32, tag="sumh")
                  nc.vector.reduce_sum(out=sumh[:], in_=t_hT[:], axis=X)
                  nc.sync.dma_start(out=d_sumh[:], in_=sumh[:])

            if hw_loop:
                with tc.For_i(0, reps):
                    body(restore=True)
            else:
                for rep in range(reps):
                    body(restore=rep > 0)

    return nc


def _get_nc(reps=1, hw_loop=False):
    key = f"nc{reps}_{hw_loop}"
    if key not in _CACHE:
        _CACHE[key] = _build_nc(reps, hw_loop)
    return _CACHE[key]


def check_waits(nc, max_waits=1, verbose=True):
    """Report instructions carrying more than `max_waits` semaphore waits."""
    bad = []
    for f in nc.m.functions:
        for bb in f.blocks:
            for ins in bb.instructions:
                si = ins.sync_info
                if si is None:
                    continue
                ow = si.on_wait or []
                if len(ow) > max_waits:
                    bad.append((ins.name, type(ins).__name__, ins.engine,
                                [w.ant_name for w in ow]))
    if verbose:
        for b in bad:
            print("MULTIWAIT:", b)
    return bad


def _shared_inputs(edge_w, edge_b, node_w, node_b):
    centers = np.linspace(0.0, VMAX, BINS).astype(np.float64)
    # groups live at 64-partition-aligned offsets (matmul base-partition rule)
    cp = 2.0 * GAMMA * centers
    cphi = cp.astype(np.float16)
    cplo = (cp - cphi.astype(np.float64)).astype(np.float16)
    cE = np.zeros((10, 64 * G), np.float64)
    cbias = np.zeros((64 * G, 1), np.float32)
    ewR = np.zeros((64 * G, NL * H), np.float32)
    for g in range(G):
        base, col = 5 * g, 64 * g
        cE[base + 0, col:col + BINS] = -GAMMA
        cE[base + 1, col:col + BINS] = -GAMMA
        cE[base + 2, col:col + BINS] = cphi
        cE[base + 3, col:col + BINS] = cplo
        cE[base + 4, col:col + BINS] = cphi
        cbias[64 * g:64 * g + BINS, 0] = -GAMMA * centers * centers
        for l in range(NL):
            ewR[64 * g:64 * g + BINS, l * H:(l + 1) * H] = edge_w[l]
    cE = cE.astype(np.float16)
    ewR = ewR.astype(ml_dtypes.bfloat16)
    # silu-softplus: device computes silu(0.5*gp + bias) -> bias = eb/2
    ebT = np.ascontiguousarray(0.5 * edge_b.T).astype(np.float32)  # [H, NL]
    nwT = np.concatenate([node_w[l] for l in range(NL)], axis=1)
    nwT = np.ascontiguousarray(nwT).astype(np.float32)           # [H, NL*H]
    nbT = np.ascontiguousarray(node_b.T).astype(np.float32)      # [H, NL]
    return dict(cE=cE, cbias=cbias, ewR=ewR, ebT=ebT, nwT=nwT, nbT=nbT)


def _hl(x):
    hi = x.astype(np.float16)
    lo = (x - hi.astype(np.float64)).astype(np.float16)
    return hi, lo


def make_in_maps(atom_types, frac_coords, lattice, mask, emb_table,
                 edge_w, edge_b, node_w, node_b):
    shared = _shared_inputs(edge_w, edge_b, node_w, node_b)
    in_maps = []
    for b in range(B):
        cart = (frac_coords[b] @ lattice[b]).astype(np.float32)  # (N, 3)
        diff = cart[:, None, :].astype(np.float64) - cart[None, :, :]
        d2e = (diff * diff).sum(-1) + 1e-6                       # (N, N)
        de = np.sqrt(d2e)
        # triangle streams (see _build_nc): A = rows 0..95 full width;
        # B = rows 96..127 full, then rows 128..255 at j in [128, 256)
        sA = (d2e[:96, :].reshape(-1), de[:96, :].reshape(-1))
        sB = (np.concatenate([d2e[96:IPG, :].reshape(-1),
                              d2e[IPG:, IPG:].reshape(-1)]),
              np.concatenate([de[96:IPG, :].reshape(-1),
                              de[IPG:, IPG:].reshape(-1)]))
        rfin = np.zeros((10, LOCG), np.float16)
        for s, (xd2, xd) in enumerate([sA, sB]):
            d2hi, d2lo = _hl(xd2)
            dhi, dlo = _hl(xd)
            base = 5 * s
            rfin[base + 0] = d2hi
            rfin[base + 1] = d2lo
            rfin[base + 2] = dhi
            rfin[base + 3] = dhi
            rfin[base + 4] = dlo
        types = np.where(mask[b], atom_types[b], 0).astype(np.int64)
        h0T = np.ascontiguousarray(emb_table[types].T).astype(np.float32)
        maskF = np.broadcast_to(
            mask[b].astype(np.float32)[None, :], (H, N)
        ).copy()
        in_maps.append(dict(rfin=rfin, h0T=h0T, maskF=maskF, **shared))
    return in_maps


def kernel(**inputs):
    from concourse.bass_utils import run_bass_kernel_spmd

    atom_types = np.asarray(inputs["atom_types"])
    frac_coords = np.asarray(inputs["frac_coords"], np.float32)
    lattice = np.asarray(inputs["lattice"], np.float32)
    mask = np.asarray(inputs["mask"]).astype(bool)
    emb_table = np.asarray(inputs["emb_table"], np.float32)
    edge_w = np.asarray(inputs["edge_w"], np.float32)
    edge_b = np.asarray(inputs["edge_b"], np.float32)
    node_w = np.asarray(inputs["node_w"], np.float32)
    node_b = np.asarray(inputs["node_b"], np.float32)
    mu_w = np.asarray(inputs["mu_w"], np.float32)
    mu_b = np.asarray(inputs["mu_b"], np.float32)
    var_w = np.asarray(inputs["var_w"], np.float32)
    var_b = np.asarray(inputs["var_b"], np.float32)

    nc = _get_nc()
    in_maps = make_in_maps(atom_types, frac_coords, lattice, mask, emb_table,
                           edge_w, edge_b, node_w, node_b)
    res = run_bass_kernel_spmd(nc, in_maps, core_ids=list(range(B)))
    sum_h = np.stack([res.results[b]["sumh"][:, 0] for b in range(B)])
    n_valid = mask.sum(1).astype(np.float32)
    g = sum_h / (n_valid[:, None] + 1e-6)
    mu = (g @ mu_w + mu_b).astype(np.float32)
    log_var = (g @ var_w + var_b).astype(np.float32)
    return mu, log_var



# revision 39
# speedup vs baseline: 1.0198x; 1.0019x over previous
"""CrystalEncoder Trainium2 kernel.

Strategy: pure data parallel — one crystal (batch element) per NeuronCore.
All O(N^2) work (RBF expansion, gated message passing) runs on-device in a
single fused Bass/Tile kernel; the host does O(N^2) *scalar* prep only
(pairwise distances in numpy, fp16 hi/lo split) plus the final projections.

Device dataflow per core (N=256 atoms, H=128, BINS=40, NL=2):
  1. RBF exponent args for all 40 bins via a K=5-per-stream fp16 matmul
     over host-supplied rows [d2hi, d2lo, dhi, dhi, dlo] paired with
     stationary rows [-g, -g, c'hi, c'lo, c'hi]  (c' = 2*gamma*c_k; hi/lo
     fp16 splitting keeps the exponent arg accurate to ~3e-3); bias
     -gamma*c_k^2 folded into the Exp activation.  rbfT bf16 resident.
  2. Triangle layout (the gate is symmetric in (i,j) since it depends
     only on d_ij): rows i<128 stored full width, rows i>=128 at half
     width (j>=128) — 49152 instead of 65536 pairs, split into two
     40-bin groups of 24576 pairs at partition offsets 0/64.
  3. Per layer: gate matmul (edge_w stationary, K=40 bf16, PSUM);
     softplus(z) ~= ln2 + 2*silu(z/2) in ONE ACT pass (the hw act tables
     have no softplus; midpoint-quadrature error < 3e-4 for |z|<1);
     the whole agg + node update collapses into a PE PSUM z-accumulation
       z[h',i] += (node_w)^T @ (h_j * gate_row_j)
     with h_j (and the 2u+ln2 affine) folded in by one DVE tensor_scalar
     per row, 2 j-rows packed per 512-col matmul into 2 accumulators.
     Half rows consume their i<128 mirror entries as strided reads from
     the retained full-row gate buffer (gtFull).
  4. Lag-D software pipeline with deferred layer tails so ACT/PE never
     head-of-line block on the fold+silu+h-update chain at layer
     boundaries; first D chunks write a gtHead side buffer to dodge the
     cross-layer WAR on gtFull.
  5. Pooling: reduce over atoms -> sum_h [H, 1] -> DRAM.
Host: g = sum_h / (n_valid + 1e-6); mu / log_var projections.

Sync discipline: this walrus build supports at most ONE semaphore wait per
instruction; the installed wait-splitter turns multi-wait instructions
into single-wait NoOp carriers, and "dep nops" pre-observe producer ticks.
"""

import numpy as np
import ml_dtypes

B, N, H, LAT, NL, BINS = 8, 256, 128, 64, 2, 40
VMAX = 8.0
GAMMA = 1.0 / (VMAX / BINS) ** 2  # 25.0

G = 2                 # 40-bin groups at partition offsets 0 / 64
IPG = N // G          # 128; also the full-width row count (i < 128)
# Triangle split: the gate is symmetric in (i, j), so rows i >= 128 only
# store j in [128, 256) (their i < 128 mirror entries are read from the
# retained full-width rows).  Pairs: 128*256 (full) + 128*128 (half) =
# 49152, split evenly into two bin-groups of LOCG pairs:
#   group A (bins at partitions 0-39):   rows  0..95, full width
#   group B (bins at partitions 64-103): rows 96..127 full width, then
#                                        rows 128..255 at half width
LOCG = 24576          # pairs per group (free size of rbfT)
NFILL = 3             # rf staging buffer fills per group stream
FILLF = LOCG // NFILL  # 8192 pairs per rf fill
ECHUNK = 2048         # pairs per Exp activation in rbf stage
CHUNK = 1024          # pairs per gate chunk (4 full rows / 8 half rows)
NCHUNK = 2 * LOCG // CHUNK  # 48 gate chunks per layer
IPC = CHUNK // N      # full-width i-rows per chunk

_CACHE = {}


def _install_wait_splitter():
    """This walrus build supports at most ONE semaphore wait per ISA
    instruction. Split every multi-wait instruction by inserting same-engine
    NoOp carriers, each holding one of the waits, immediately before it.
    Semantics are preserved: the engine executes its stream in order, so all
    original wait conditions still hold before the instruction runs."""
    import bass_rust
    import concourse.tile as tile
    from concourse import mybir

    if getattr(tile.TileContext, "_wait_split_installed", False):
        return
    orig = tile.TileContext._lower_ordered_insts
    counter = [0]

    def patched(self, ordered):
        for insts in ordered.values():
            newl = []
            for inst in insts:
                si = inst.sync_info
                ow = list(si.on_wait) if (si is not None and si.on_wait) else []
                if len(ow) > 1 and inst.engine != mybir.EngineType.Unassigned:
                    for w in ow[:-1]:
                        counter[0] += 1
                        nop = bass_rust.InstNoOp(
                            name=f"wsplit_{counter[0]}", ins=[], outs=[]
                        )
                        nop.engine = inst.engine
                        nop.sync_info = bass_rust.SyncInfo(
                            on_wait=[w], on_update=[]
                        )
                        newl.append(nop)
                    inst.sync_info = bass_rust.SyncInfo(
                        on_wait=[ow[-1]], on_update=list(si.on_update or [])
                    )
                newl.append(inst)
            insts[:] = newl
        return orig(self, ordered)

    tile.TileContext._lower_ordered_insts = patched

    def patched_dab(self, tick_clock, wait_clock):
        # Reimplementation of _drain_and_barrier: the kernel-tail drain
        # otherwise carries one wait per proc (11 here). Emit single-wait SP
        # nop carriers covering the global clock, then a bare drain.
        from concourse.vector_clock import ScopedClock

        probe = self.nc.sync.nop()
        wait_clock.add_sem_waits(
            probe.ins, ScopedClock({None: tick_clock.global_clock})
        )
        si = probe.ins.sync_info
        ow = list(si.on_wait) if (si is not None and si.on_wait) else []
        if len(ow) > 1:
            probe.ins.sync_info = bass_rust.SyncInfo(
                on_wait=[ow[0]], on_update=list(si.on_update or [])
            )
            for w in ow[1:]:
                n2 = self.nc.sync.nop()
                n2.ins.sync_info = bass_rust.SyncInfo(on_wait=[w], on_update=[])
        self.nc.sync.drain()
        self.nc.all_engine_barrier()
        popped = self.nc._tile_sem_poison_stack.pop()
        assert popped is self._sem_poison
        self.nc.clear_and_free_semaphores(list(self.sems.allocated().values()))
        self.nc.all_engine_barrier()

    tile.TileContext._drain_and_barrier = patched_dab
    tile.TileContext._wait_split_installed = True


def _build_nc(reps=1, hw_loop=False):
    import concourse.bass as bass
    import concourse.tile as tile
    from concourse import mybir

    _install_wait_splitter()

    F32 = mybir.dt.float32
    BF16 = mybir.dt.bfloat16
    AF = mybir.ActivationFunctionType
    X = mybir.AxisListType.X
    POOL = mybir.EngineType.Pool

    nc = bass.Bass("TRN2", target_bir_lowering=False, debug=False)

    def dep_nop(engine, aps):
        """Engine-local nop reading `aps`: pulls their producers' ticks into
        the engine's observed clock so later real instructions need at most
        one new semaphore wait."""
        nop = engine.nop(hint="dep").ins
        nop.ins = [engine.lower_ap(ap) for ap in aps]
        return nop

    FP16 = mybir.dt.float16

    # rfin rows (host-computed, fp16 hi/lo split so the RBF-argument
    # matmul runs in fp16 at full PE rate with ~3e-3 absolute accuracy):
    #   per stream s in {A, B}: [d2hi, d2lo, dhi, dhi, dlo]
    # paired with cE rows [-g, -g, c'hi, c'lo, c'hi]  (c' = 2*gamma*c_k)
    d_rfin = nc.dram_tensor("rfin", [10, LOCG], FP16, kind="ExternalInput")
    d_h0T = nc.dram_tensor("h0T", [H, N], F32, kind="ExternalInput")
    d_maskF = nc.dram_tensor("maskF", [H, N], F32, kind="ExternalInput")
    d_cE = nc.dram_tensor("cE", [10, 64 * G], FP16, kind="ExternalInput")
    d_cbias = nc.dram_tensor("cbias", [64 * G, 1], F32, kind="ExternalInput")
    d_ewR = nc.dram_tensor("ewR", [64 * G, NL * H], BF16, kind="ExternalInput")
    d_ebT = nc.dram_tensor("ebT", [H, NL], F32, kind="ExternalInput")
    d_nwT = nc.dram_tensor("nwT", [H, NL * H], F32, kind="ExternalInput")
    d_nbT = nc.dram_tensor("nbT", [H, NL], F32, kind="ExternalInput")
    d_sumh = nc.dram_tensor("sumh", [H, 1], F32, kind="ExternalOutput")

    with tile.TileContext(nc) as tc:
        with tc.tile_pool(name="consts", bufs=1) as consts:
            kw = dict(forced_dma_engine=POOL)
            # stage-2-critical consts ride the fast HWDGE queue ahead of
            # the first rfin fill; the rest load in parallel on SWDGE
            kws = dict(forced_dma_engine=mybir.EngineType.SP)
            t_cE = consts.tile_from(d_cE[:], **kws)
            t_cbias = consts.tile_from(d_cbias[:], **kws)
            t_hT = consts.tile_from(d_h0T[:], **kw)
            t_maskF = consts.tile_from(d_maskF[:], **kw)
            t_ewR = consts.tile_from(d_ewR[:], **kw)
            t_ebT = consts.tile_from(d_ebT[:], **kw)
            t_nwT = consts.tile_from(d_nwT[:], **kw)
            t_nbT = consts.tile_from(d_nbT[:], **kw)

            rbfT = consts.tile([64 * G, LOCG], BF16)

            # every engine pre-observes the (single) DMA proc at its max tick
            dep_nop(nc.tensor, [t_cE[:], t_ewR[:], t_nwT[:]])
            dep_nop(nc.scalar, [t_cbias[:], t_ebT[:], t_nbT[:]])
            dep_nop(nc.vector, [t_hT[:], t_maskF[:]])

            h00 = consts.tile([H, N], mybir.dt.float32, tag="h00")
            nc.vector.tensor_copy(h00[:], t_hT[:])
            t_nwB = consts.tile([H, NL * H], BF16, tag="nwB")
            nc.vector.tensor_copy(t_nwB[:], t_nwT[:])

            def body(restore):
              if restore:
                # restore initial h (body updates t_hT in place)
                nc.vector.tensor_copy(t_hT[:], h00[:])
              # ---- stage 2: resident RBF table from host distances ----
              with tc.tile_pool(name="rfp", bufs=2) as rfp, \
                   tc.tile_pool(name="geop", bufs=2, space="PSUM") as geop:
                  for hf in range(NFILL):
                      rf = rfp.tile([10, FILLF], FP16, tag="rf")
                      nc.sync.dma_start(
                          out=rf[:],
                          in_=d_rfin[:, hf * FILLF:(hf + 1) * FILLF])
                      dep_nop(nc.tensor, [rf[:]])
                      for cc in range(FILLF // ECHUNK):
                          e = geop.tile([64 * G, ECHUNK], F32, tag="ps")
                          for s4 in range(ECHUNK // 512):
                              f0 = cc * ECHUNK + s4 * 512
                              nc.tensor.matmul(
                                  e[:, s4 * 512:(s4 + 1) * 512],
                                  t_cE[:], rf[:, f0:f0 + 512],
                                  start=True, stop=True,
                              )
                          o0 = hf * FILLF + cc * ECHUNK
                          nc.scalar.activation(
                              rbfT[:, o0:o0 + ECHUNK], e[:], AF.Exp,
                              bias=t_cbias[:],
                          )

              # ---- stage 3: message-passing layers ----
              # The gate is symmetric: gate[h,i,j] == gate[h,j,i] (it only
              # depends on d_ij and per-h weights).  So chunk row r, which
              # holds gate[h, i=i0+r, all j], is ALSO the column j=i0+r over
              # all i.  That lets the whole agg+node-update collapse into a
              # PSUM accumulation on PE:
              #   z[h',i] = sum_j nw[h,h']^T @ (h_j * gate[h, j, i])
              # with h_j folded in by one DVE tensor_scalar per row.  No DVE
              # reduce at all; the node matmul disappears into the z-accum.
              #
              # softplus(z) ~= ln2 + z*sigmoid(z/2) = ln2 + 2*silu(z/2)
              # (midpoint quadrature of softplus' = sigmoid; |err| < 3e-4
              # for |z| < 1, and |z| < 0.5 here).  Silu IS in the hw act
              # tables (softplus is not), and the affine 2u+ln2 folds into
              # the per-row tensor_scalar:
              #   gth = u_row * (2 h_j) + (ln2 h_j)
              # Triangle: rows i>=128 are stored at half width; their
              # i<128 mirror entries are strided reads from gtFull.
              # Lag-D software pipeline: chunk c is produced at step c and
              # consumed at step c+D, and each layer's tail ops (fold,
              # silu, h update) are emitted D chunks INTO the next layer's
              # production so neither ACT nor PE head-of-line blocks on
              # the layer transition.  The first D chunks of each layer
              # write a small gtHead buffer (relayed to gtFull mid-layer)
              # so their softplus doesn't WAR-stall on the previous
              # layer's mirror reads of gtFull.
              D = 5
              with tc.tile_pool(name="lay", bufs=1) as lay, \
                   tc.tile_pool(name="work", bufs=3) as work, \
                   tc.tile_pool(name="wgt", bufs=D + 2) as wgt, \
                   tc.tile_pool(name="gpp", bufs=3, space="PSUM") as gpp, \
                   tc.tile_pool(name="zpp", bufs=2, space="PSUM") as zpp:
                  gtFull = lay.tile([H, IPG * N], BF16, tag="gtFull")
                  gtF3 = gtFull[:].rearrange("p (i c) -> p i c", c=N)
                  gtHead = lay.tile([H, D * CHUNK], BF16, tag="gtHead")
                  gtH3 = gtHead[:].rearrange("p (i c) -> p i c", c=N)

                  def chunk_info(c):
                      if c < 24:            # group A, full rows 0..95
                          return 0, c * CHUNK, 4 * c, True
                      if c < 32:            # group B, full rows 96..127
                          cb = c - 24
                          return 1, cb * CHUNK, 96 + 4 * cb, True
                      ch = c - 32           # group B, half rows 128..255
                      return 1, 8 * CHUNK + ch * CHUNK, 128 + 8 * ch, False

                  def make_layer_end(l, zp):
                      def go():
                          # fold the 2 accumulators (PSUM allows only one
                          # PSUM operand per DVE op: copy then add)
                          zf0 = lay.tile([H, N], F32, tag=f"zf0_{l}")
                          nc.vector.tensor_copy(zf0[:], zp[:, 0:N])
                          zf = lay.tile([H, N], F32, tag=f"zf_{l}")
                          nc.vector.tensor_add(zf[:], zf0[:],
                                               zp[:, N:2 * N])
                          sl = lay.tile([H, N], F32, tag=f"sil{l}")
                          nc.scalar.activation(
                              sl[:], zf[:], AF.Silu,
                              bias=t_nbT[:, l:l + 1],
                          )
                          h2 = lay.tile([H, N], F32, tag=f"h2_{l}")
                          nc.vector.tensor_add(h2[:], t_hT[:], sl[:])
                          nc.vector.tensor_mul(t_hT[:], h2[:], t_maskF[:])
                      return go

                  pend = None
                  for l in range(NL):
                      zp = zpp.tile([H, 2 * N], F32, tag="zp")
                      th2 = thl2 = None
                      gts = [None] * NCHUNK
                      for c in range(NCHUNK + D):
                          if c < NCHUNK:
                              # produce gate chunk c:  u = silu(z/2 + eb/2)
                              g, col0, i0, full = chunk_info(c)
                              gp = gpp.tile([H, CHUNK], F32, tag="gp")
                              for s4 in range(CHUNK // 512):
                                  nc.tensor.matmul(
                                      gp[:, s4 * 512:(s4 + 1) * 512],
                                      t_ewR[64 * g:64 * g + BINS,
                                            l * H:(l + 1) * H],
                                      rbfT[64 * g:64 * g + BINS,
                                           col0 + s4 * 512:
                                           col0 + (s4 + 1) * 512],
                                      start=True, stop=True,
                                  )
                              if full and c < D:
                                  dest = gtHead[:, c * CHUNK:
                                                (c + 1) * CHUNK]
                              elif full:
                                  dest = gtFull[:, i0 * N:i0 * N + CHUNK]
                              else:
                                  gt = wgt.tile([H, CHUNK], BF16,
                                                tag="gt")
                                  gts[c] = gt
                                  dest = gt[:]
                              nc.scalar.activation(
                                  dest, gp[:], AF.Silu,
                                  bias=t_ebT[:, l:l + 1], scale=0.5,
                              )
                          if c == D:
                              # previous layer's tail, then this layer's
                              # per-partition gate-affine factors
                              if pend is not None:
                                  pend()
                              th2 = lay.tile([H, N], F32, tag=f"th2_{l}")
                              nc.vector.tensor_scalar_mul(
                                  th2[:], t_hT[:], 2.0)
                              thl2 = lay.tile([H, N], F32,
                                              tag=f"thl2_{l}")
                              nc.vector.tensor_scalar_mul(
                                  thl2[:], t_hT[:], 0.6931471805599453)
                              # relay head chunks into gtFull for the
                              # half-phase mirror reads
                              for k in range(D):
                                  nc.vector.tensor_copy(
                                      gtFull[:, k * CHUNK:
                                             (k + 1) * CHUNK],
                                      gtHead[:, k * CHUNK:
                                             (k + 1) * CHUNK])
                          if c >= D:
                              cc = c - D
                              g, col0, i0, full = chunk_info(cc)
                              nwl = t_nwB[:, l * H:(l + 1) * H]
                              ts = dict(op0=mybir.AluOpType.mult,
                                        op1=mybir.AluOpType.add)
                              if full:
                                  src3 = gtH3 if cc < D else gtF3
                                  g4 = work.tile([H, 4 * N], BF16,
                                                 tag="g4")
                                  for r in range(IPC):
                                      j = i0 + r
                                      nc.vector.tensor_scalar(
                                          g4[:, r * N:(r + 1) * N],
                                          src3[:, j, :],
                                          th2[:, j:j + 1],
                                          thl2[:, j:j + 1], **ts,
                                      )
                                  for s4 in range(2):
                                      nc.tensor.matmul(
                                          zp[:],
                                          nwl,
                                          g4[:, s4 * 512:(s4 + 1) * 512],
                                          start=(cc == 0 and s4 == 0),
                                          stop=False,
                                      )
                              else:
                                  gtc = gts[cc]
                                  for half in range(2):
                                      g4 = work.tile([H, 4 * N], BF16,
                                                     tag="g4")
                                      for s in range(4):
                                          r = 4 * half + s
                                          j = i0 + r
                                          # piece A: stored half row ->
                                          # z cols [128, 256) of slot s
                                          nc.vector.tensor_scalar(
                                              g4[:, s * N + IPG:
                                                 (s + 1) * N],
                                              gtc[:, r * IPG:
                                                  (r + 1) * IPG],
                                              th2[:, j:j + 1],
                                              thl2[:, j:j + 1], **ts,
                                          )
                                          # piece B: mirrored column j of
                                          # full rows -> z cols [0, 128)
                                          nc.vector.tensor_scalar(
                                              g4[:, s * N:s * N + IPG],
                                              gtF3[:, :, j],
                                              th2[:, j:j + 1],
                                              thl2[:, j:j + 1], **ts,
                                          )
                                      for s4 in range(2):
                                          nc.tensor.matmul(
                                              zp[:],
                                              nwl,
                                              g4[:, s4 * 512:
                                                 (s4 + 1) * 512],
                                              start=False,
                                              stop=(cc == NCHUNK - 1
                                                    and half == 1
                                                    and s4 == 1),
                                          )
                                  gts[cc] = None
                      pend = make_layer_end(l, zp)
                  pend()

                  sumh = lay.tile([H, 1], F32, tag="sumh")
                  nc.vector.reduce_sum(out=sumh[:], in_=t_hT[:], axis=X)
                  nc.sync.dma_start(out=d_sumh[:], in_=sumh[:])

            if hw_loop:
                with tc.For_i(0, reps):
                    body(restore=True)
            else:
                for rep in range(reps):
                    body(restore=rep > 0)

    return nc


def _get_nc(reps=1, hw_loop=False):
    key = f"nc{reps}_{hw_loop}"
    if key not in _CACHE:
        _CACHE[key] = _build_nc(reps, hw_loop)
    return _CACHE[key]


def check_waits(nc, max_waits=1, verbose=True):
    """Report instructions carrying more than `max_waits` semaphore waits."""
    bad = []
    for f in nc.m.functions:
        for bb in f.blocks:
            for ins in bb.instructions:
                si = ins.sync_info
                if si is None:
                    continue
                ow = si.on_wait or []
                if len(ow) > max_waits:
                    bad.append((ins.name, type(ins).__name__, ins.engine,
                                [w.ant_name for w in ow]))
    if verbose:
        for b in bad:
            print("MULTIWAIT:", b)
    return bad


def _shared_inputs(edge_w, edge_b, node_w, node_b):
    centers = np.linspace(0.0, VMAX, BINS).astype(np.float64)
    # groups live at 64-partition-aligned offsets (matmul base-partition rule)
    cp = 2.0 * GAMMA * centers
    cphi = cp.astype(np.float16)
    cplo = (cp - cphi.astype(np.float64)).astype(np.float16)
    cE = np.zeros((10, 64 * G), np.float64)
    cbias = np.zeros((64 * G, 1), np.float32)
    ewR = np.zeros((64 * G, NL * H), np.float32)
    for g in range(G):
        base, col = 5 * g, 64 * g
        cE[base + 0, col:col + BINS] = -GAMMA
        cE[base + 1, col:col + BINS] = -GAMMA
        cE[base + 2, col:col + BINS] = cphi
        cE[base + 3, col:col + BINS] = cplo
        cE[base + 4, col:col + BINS] = cphi
        cbias[64 * g:64 * g + BINS, 0] = -GAMMA * centers * centers
        for l in range(NL):
            ewR[64 * g:64 * g + BINS, l * H:(l + 1) * H] = edge_w[l]
    cE = cE.astype(np.float16)
    ewR = ewR.astype(ml_dtypes.bfloat16)
    # silu-softplus: device computes silu(0.5*gp + bias) -> bias = eb/2
    ebT = np.ascontiguousarray(0.5 * edge_b.T).astype(np.float32)  # [H, NL]
    nwT = np.concatenate([node_w[l] for l in range(NL)], axis=1)
    nwT = np.ascontiguousarray(nwT).astype(np.float32)           # [H, NL*H]
    nbT = np.ascontiguousarray(node_b.T).astype(np.float32)      # [H, NL]
    return dict(cE=cE, cbias=cbias, ewR=ewR, ebT=ebT, nwT=nwT, nbT=nbT)


def _hl(x):
    hi = x.astype(np.float16)
    lo = (x - hi.astype(np.float64)).astype(np.float16)
    return hi, lo


def make_in_maps(atom_types, frac_coords, lattice, mask, emb_table,
                 edge_w, edge_b, node_w, node_b):
    shared = _shared_inputs(edge_w, edge_b, node_w, node_b)
    in_maps = []
    for b in range(B):
        cart = (frac_coords[b] @ lattice[b]).astype(np.float32)  # (N, 3)
        diff = cart[:, None, :].astype(np.float64) - cart[None, :, :]
        d2e = (diff * diff).sum(-1) + 1e-6                       # (N, N)
        de = np.sqrt(d2e)
        # triangle streams (see _build_nc): A = rows 0..95 full width;
        # B = rows 96..127 full, then rows 128..255 at j in [128, 256)
        sA = (d2e[:96, :].reshape(-1), de[:96, :].reshape(-1))
        sB = (np.concatenate([d2e[96:IPG, :].reshape(-1),
                              d2e[IPG:, IPG:].reshape(-1)]),
              np.concatenate([de[96:IPG, :].reshape(-1),
                              de[IPG:, IPG:].reshape(-1)]))
        rfin = np.zeros((10, LOCG), np.float16)
        for s, (xd2, xd) in enumerate([sA, sB]):
            d2hi, d2lo = _hl(xd2)
            dhi, dlo = _hl(xd)
            base = 5 * s
            rfin[base + 0] = d2hi
            rfin[base + 1] = d2lo
            rfin[base + 2] = dhi
            rfin[base + 3] = dhi
            rfin[base + 4] = dlo
        types = np.where(mask[b], atom_types[b], 0).astype(np.int64)
        h0T = np.ascontiguousarray(emb_table[types].T).astype(np.float32)
        maskF = np.broadcast_to(
            mask[b].astype(np.float32)[None, :], (H, N)
        ).copy()
        in_maps.append(dict(rfin=rfin, h0T=h0T, maskF=maskF, **shared))
    return in_maps


def kernel(**inputs):
    from concourse.bass_utils import run_bass_kernel_spmd

    atom_types = np.asarray(inputs["atom_types"])
    frac_coords = np.asarray(inputs["frac_coords"], np.float32)
    lattice = np.asarray(inputs["lattice"], np.float32)
    mask = np.asarray(inputs["mask"]).astype(bool)
    emb_table = np.asarray(inputs["emb_table"], np.float32)
    edge_w = np.asarray(inputs["edge_w"], np.float32)
    edge_b = np.asarray(inputs["edge_b"], np.float32)
    node_w = np.asarray(inputs["node_w"], np.float32)
    node_b = np.asarray(inputs["node_b"], np.float32)
    mu_w = np.asarray(inputs["mu_w"], np.float32)
    mu_b = np.asarray(inputs["mu_b"], np.float32)
    var_w = np.asarray(inputs["var_w"], np.float32)
    var_b = np.asarray(inputs["var_b"], np.float32)

    nc = _get_nc()
    in_maps = make_in_maps(atom_types, frac_coords, lattice, mask, emb_table,
                           edge_w, edge_b, node_w, node_b)
    res = run_bass_kernel_spmd(nc, in_maps, core_ids=list(range(B)))
    sum_h = np.stack([res.results[b]["sumh"][:, 0] for b in range(B)])
    n_valid = mask.sum(1).astype(np.float32)
    g = sum_h / (n_valid[:, None] + 1e-6)
    mu = (g @ mu_w + mu_b).astype(np.float32)
    log_var = (g @ var_w + var_b).astype(np.float32)
    return mu, log_var

